# revision 1
# baseline (speedup 1.0000x reference)
"""Trainium2 Bass kernel for nn_AttnAdapter: GQA attention with RoPE,
region-based enhance/suppress score scaling, causal mask, o_proj.

Sharding: tensor-parallel over heads across 8 NeuronCores. Core d holds
q-heads 4d..4d+3 (wq rows), kv-head d (wk/wv rows), and wo columns
512d..512(d+1). Each core computes a full [S, D] partial of the output;
the host sums the 8 partials (the TP all-reduce, done at unshard time).

All on-chip matmuls run in float32r (full PE rate for N>=512) over
transposed layouts so no on-device transposes are needed except V
(16 PE 128x128 transposes). xT and the output are host-tiled so every
DMA moves one fully contiguous 256KB block.
"""

import math

import numpy as np

# ---- problem constants (hardcoded; kernel.py must be self-contained) ----
S = 2048          # sequence length
D = 4096          # model dim
HD = 128          # head dim
NCORES = 8
QH = 4            # q heads per core
SYS_LEN, IMG_LEN = 35, 576
BOUND = SYS_LEN + IMG_LEN          # 611
ENH, SUP = 1.5, 0.5
ROPE_BASE = 10000.0

J = 4             # sq tiles of 512
NSK = 16          # sk tiles of 128
DCH = 32          # D chunks of 128

_CACHE = {}


def _host_constants():
    inv_freq = 1.0 / (ROPE_BASE ** (np.arange(0, HD, 2, dtype=np.float32) / HD))
    pos = np.arange(S, dtype=np.float32)
    freqs = pos[:, None] * inv_freq[None, :]              # [S, 64]
    emb = np.concatenate([freqs, freqs], axis=-1)         # [S, 128]
    cosT = np.ascontiguousarray(np.cos(emb).T.astype(np.float32))  # [128, S]
    sinT = np.ascontiguousarray(np.sin(emb).T.astype(np.float32))

    # rotate_half as a matmul: rot = R @ q (in [hd, s] layout).
    # matmul(out, lhsT, rhs) = lhsT.T @ rhs, so feed RT = R.T.
    RT = np.zeros((HD, HD), dtype=np.float32)
    half = HD // 2
    for c in range(half):
        RT[c + half, c] = -1.0      # rot[c] = -q[c+64]
    for c in range(half, HD):
        RT[c - half, c] = 1.0       # rot[c] = q[c-64]

    ident = np.eye(HD, dtype=np.float32)

    # Diagonal-tile causal masks, T layout [sk 128, sq 512]:
    # tile (i=4j+delta, j): valid (keep) iff sq >= sk  <=>  f >= 128*delta + p
    masks = np.zeros((HD, 4 * 512), dtype=np.float32)
    p = np.arange(128)[:, None]
    f = np.arange(512)[None, :]
    for delta in range(4):
        masks[:, delta * 512:(delta + 1) * 512] = (f >= 128 * delta + p)

    # key_scale in partition layout per sk-tile: ksT[p, i] = scale(128*i+p)
    kpos = np.arange(S)
    key_scale = np.where(kpos < SYS_LEN, SUP,
                         np.where(kpos < BOUND, ENH, 1.0)).astype(np.float32)
    ksT = np.ascontiguousarray(key_scale[:5 * 128].reshape(5, 128).T)  # [128, 5]

    ones_col = np.ones((HD, 1), dtype=np.float32)
    return dict(cosT=cosT, sinT=sinT, rmat=RT, ident=ident, masks=masks,
                ksT=ksT, ones_col=ones_col)


def _build_bass():
    import concourse.bass as bass
    import concourse.mybir as mybir
    from concourse.tile import TileContext
    from contextlib import ExitStack

    f32 = mybir.dt.float32
    f32r = mybir.dt.float32r

    nc = bass.Bass()
    # xTt[d, j, p, f] = x.T[128d+p, 512j+f] -- each (d,j) tile contiguous
    xTt = nc.dram_tensor("xTt", [DCH, J, 128, 512], f32r, kind="ExternalInput")
    wqT = nc.dram_tensor("wqT", [D, QH * HD], f32r, kind="ExternalInput")
    wkT = nc.dram_tensor("wkT", [D, HD], f32r, kind="ExternalInput")
    wvT = nc.dram_tensor("wvT", [D, HD], f32r, kind="ExternalInput")
    woT = nc.dram_tensor("woT", [QH * HD, D], f32r, kind="ExternalInput")
    cosT_d = nc.dram_tensor("cosT", [HD, S], f32, kind="ExternalInput")
    sinT_d = nc.dram_tensor("sinT", [HD, S], f32, kind="ExternalInput")
    rmat_d = nc.dram_tensor("rmat", [HD, HD], f32r, kind="ExternalInput")
    ident_d = nc.dram_tensor("ident", [HD, HD], f32, kind="ExternalInput")
    masks_d = nc.dram_tensor("masks", [HD, 4 * 512], f32, kind="ExternalInput")
    ksT_d = nc.dram_tensor("ksT", [HD, 5], f32, kind="ExternalInput")
    onesc_d = nc.dram_tensor("ones_col", [HD, 1], f32r, kind="ExternalInput")
    onesr_d = nc.dram_tensor("ones_row", [1, HD], f32r, kind="ExternalInput")
    # out_t[t, n, p, f] = out[128t+p, 512n+f] -- contiguous per tile
    out = nc.dram_tensor("out", [NSK, 8, 128, 512], f32, kind="ExternalOutput")

    EXP = mybir.ActivationFunctionType.Exp

    with TileContext(nc) as tc, ExitStack() as ctx:
        const = ctx.enter_context(tc.tile_pool(name="const", bufs=1))
        cosT = const.tile([HD, S], f32)
        nc.sync.dma_start(cosT[:], cosT_d[:, :])
        sinT = const.tile([HD, S], f32)
        nc.sync.dma_start(sinT[:], sinT_d[:, :])
        rmat = const.tile([HD, HD], f32r)
        nc.sync.dma_start(rmat[:], rmat_d[:, :])
        ident = const.tile([HD, HD], f32)
        nc.sync.dma_start(ident[:], ident_d[:, :])
        masks = const.tile([HD, 4 * 512], f32)
        nc.sync.dma_start(masks[:], masks_d[:, :])
        ksT = const.tile([HD, 5], f32)
        nc.sync.dma_start(ksT[:], ksT_d[:, :])
        ones_col = const.tile([HD, 1], f32r)
        nc.sync.dma_start(ones_col[:], onesc_d[:, :])
        ones_row = const.tile([1, HD], f32r)
        nc.sync.dma_start(ones_row[:], onesr_d[:, :])

        persist = ctx.enter_context(tc.tile_pool(name="persist", bufs=1))
        qrot = [persist.tile([HD, S], f32r, name=f"qrot{m}") for m in range(QH)]
        krot = persist.tile([HD, S], f32r)
        vnat = persist.tile([HD, NSK * HD], f32r)  # tile i at cols i*128
        attn = [persist.tile([HD, S], f32r, name=f"attn{h}") for h in range(QH)]

        # ---------------- Phase A: projections + RoPE + V transpose --------
        with tc.tile_pool(name="xw", bufs=4) as xw, \
             tc.tile_pool(name="accp", bufs=1, space="PSUM") as accp, \
             tc.tile_pool(name="ropep", bufs=2, space="PSUM") as ropep, \
             tc.tile_pool(name="stage", bufs=3) as stage:
            for j in range(J):
                sq = slice(j * 512, (j + 1) * 512)
                accs = [accp.tile([128, 512], f32, name=f"acc{m}") for m in range(6)]
                for d in range(DCH):
                    dd = slice(d * 128, (d + 1) * 128)
                    xt = xw.tile([128, 512], f32r, tag="xt")
                    nc.sync.dma_start(xt[:], xTt[d, j])
                    wq_t = xw.tile([128, 512], f32r, tag="wq")
                    nc.sync.dma_start(wq_t[:], wqT[dd, :])
                    wkv_t = xw.tile([128, 256], f32r, tag="wkv")
                    nc.sync.dma_start(wkv_t[:, 0:128], wkT[dd, :])
                    nc.sync.dma_start(wkv_t[:, 128:256], wvT[dd, :])
                    st = (d == 0)
                    sp = (d == DCH - 1)
                    for m in range(QH):
                        nc.tensor.matmul(accs[m][:], wq_t[:, m * 128:(m + 1) * 128],
                                         xt[:], start=st, stop=sp)
                    nc.tensor.matmul(accs[4][:], wkv_t[:, 0:128], xt[:],
                                     start=st, stop=sp)
                    nc.tensor.matmul(accs[5][:], wkv_t[:, 128:256], xt[:],
                                     start=st, stop=sp)

                # RoPE for q tiles and k tile; PSUM released by the ACT copy
                for m in range(5):
                    dst = qrot[m][:, sq] if m < QH else krot[:, sq]
                    q_sb = stage.tile([128, 512], f32r, tag="q_sb")
                    nc.scalar.copy(q_sb[:], accs[m][:])
                    rot_ps = ropep.tile([128, 512], f32, tag="rope_ps")
                    nc.tensor.matmul(rot_ps[:], rmat[:], q_sb[:],
                                     start=True, stop=True)
                    t1 = stage.tile([128, 512], f32, tag="t1")
                    nc.vector.tensor_mul(t1[:], q_sb[:], cosT[:, sq])
                    t2 = stage.tile([128, 512], f32, tag="t2")
                    nc.vector.tensor_mul(t2[:], rot_ps[:], sinT[:, sq])
                    nc.vector.tensor_add(dst, t1[:], t2[:])

                # V: copy to SBUF, transpose 128x128 blocks into vnat
                v_sb = stage.tile([128, 512], f32, tag="v_sb")
                nc.scalar.copy(v_sb[:], accs[5][:])
                for b in range(4):
                    i = 4 * j + b
                    vt_ps = ropep.tile([128, 512], f32, tag="rope_ps")
                    nc.tensor.transpose(vt_ps[:, 0:128],
                                        v_sb[:, b * 128:(b + 1) * 128], ident[:])
                    nc.vector.tensor_copy(vnat[:, i * 128:(i + 1) * 128],
                                          vt_ps[:, 0:128])

        # woT loads issued here so they prefetch during phase B
        wo_sb = ctx.enter_context(tc.tile_pool(name="wo_sb", bufs=1))
        wo_t = [wo_sb.tile([128, D], f32r, name=f"wo{h}") for h in range(QH)]
        for h in range(QH):
            nc.sync.dma_start(wo_t[h][:], woT[h * 128:(h + 1) * 128, :])

        # ---------------- Phase B: attention ------------------------------
        with tc.tile_pool(name="att_sb", bufs=4) as att_sb, \
             tc.tile_pool(name="sp", bufs=2, space="PSUM") as sp, \
             tc.tile_pool(name="avp", bufs=2, space="PSUM") as avp, \
             tc.tile_pool(name="dnp", bufs=2, space="PSUM") as dnp, \
             tc.tile_pool(name="rbp", bufs=2, space="PSUM") as rbp, \
             tc.tile_pool(name="nrm", bufs=3) as nrm:
            for j in range(J):
                sq = slice(j * 512, (j + 1) * 512)
                ni = 4 * j + 4            # sk tiles 0..4j+3 are live
                for h in range(QH):
                    acc_av = avp.tile([128, 512], f32, tag="av")
                    acc_dn = dnp.tile([1, 512], f32, tag="dn")
                    for i in range(ni):
                        s_ps = sp.tile([128, 512], f32, tag="s")
                        nc.tensor.matmul(s_ps[:], krot[:, i * 128:(i + 1) * 128],
                                         qrot[h][:, sq], start=True, stop=True)
                        if i < 5:
                            # region enhance/suppress for sq >= BOUND
                            c0 = 0 if j >= 2 else (BOUND - 512 if j == 1 else None)
                            if c0 is not None:
                                nc.vector.tensor_scalar_mul(
                                    s_ps[:, c0:512], s_ps[:, c0:512],
                                    ksT[:, i:i + 1])
                        e_sb = att_sb.tile([128, 512], f32r, tag="e")
                        nc.scalar.activation(e_sb[:], s_ps[:], EXP)
                        delta = i - 4 * j
                        if delta >= 0:
                            nc.vector.tensor_mul(
                                e_sb[:], e_sb[:],
                                masks[:, delta * 512:(delta + 1) * 512])
                        st = (i == 0)
                        sp_l = (i == ni - 1)
                        nc.tensor.matmul(acc_dn[:], ones_col[:], e_sb[:],
                                         start=st, stop=sp_l)
                        nc.tensor.matmul(acc_av[:], vnat[:, i * 128:(i + 1) * 128],
                                         e_sb[:], start=st, stop=sp_l)
                    # softmax denom -> 1/x = exp(-ln(x)) on ACT -> K=1 bcast
                    lrec = nrm.tile([1, 512], f32, tag="lrec")
                    nc.scalar.activation(lrec[:], acc_dn[:],
                                         mybir.ActivationFunctionType.Ln)
                    rec2 = nrm.tile([1, 512], f32r, tag="rec2")
                    nc.scalar.activation(rec2[:], lrec[:], EXP, scale=-1.0)
                    rb_ps = rbp.tile([128, 512], f32, tag="rb")
                    nc.tensor.matmul(rb_ps[:], ones_row[:], rec2[:],
                                     start=True, stop=True)
                    rb_sb = nrm.tile([128, 512], f32, tag="rb_sb")
                    nc.scalar.copy(rb_sb[:], rb_ps[:])
                    nc.vector.tensor_mul(attn[h][:, sq], acc_av[:], rb_sb[:])

        # ---------------- Phase C: o_proj ---------------------------------
        with tc.tile_pool(name="op", bufs=4, space="PSUM") as op, \
             tc.tile_pool(name="ost", bufs=4) as ost:
            for t in range(NSK):
                ts_ = slice(t * 128, (t + 1) * 128)
                for n in range(8):
                    o_ps = op.tile([128, 512], f32, tag="o")
                    for h in range(QH):
                        nc.tensor.matmul(o_ps[:], attn[h][:, ts_],
                                         wo_t[h][:, n * 512:(n + 1) * 512],
                                         start=(h == 0), stop=(h == QH - 1))
                    o_sb = ost.tile([128, 512], f32, tag="o_sb")
                    nc.any.tensor_copy(o_sb[:], o_ps[:])
                    nc.sync.dma_start(out[t, n], o_sb[:])

    # Split multi-wait instructions (self-loading f32r matmuls allow only
    # one sync wait) onto standalone EventSemaphore instructions.
    import bass_rust
    bass_rust.generate_event_semaphores(nc)
    return nc


def _get_compiled():
    if "nc" not in _CACHE:
        _CACHE["nc"] = _build_bass()
        _CACHE["const"] = _host_constants()
    return _CACHE["nc"], _CACHE["const"]


def kernel(hidden_states, wq, wk, wv, wo, _trace=False):
    from concourse.bass_utils import run_bass_kernel_spmd

    nc, cst = _get_compiled()

    x = np.asarray(hidden_states, dtype=np.float32).reshape(S, D)
    xT = np.ascontiguousarray(x.T)                       # [D, S]
    xTt = np.ascontiguousarray(
        xT.reshape(DCH, 128, J, 512).transpose(0, 2, 1, 3))  # [DCH,J,128,512]
    wq = np.asarray(wq, dtype=np.float32)
    wk = np.asarray(wk, dtype=np.float32)
    wv = np.asarray(wv, dtype=np.float32)
    wo = np.asarray(wo, dtype=np.float32)
    scale = 1.0 / math.sqrt(HD)

    in_maps = []
    for d in range(NCORES):
        wq_d = wq[d * QH * HD:(d + 1) * QH * HD] * scale      # [512, D]
        in_maps.append({
            "xTt": xTt,
            "wqT": np.ascontiguousarray(wq_d.T),
            "wkT": np.ascontiguousarray(wk[d * HD:(d + 1) * HD].T),
            "wvT": np.ascontiguousarray(wv[d * HD:(d + 1) * HD].T),
            "woT": np.ascontiguousarray(wo[:, d * QH * HD:(d + 1) * QH * HD].T),
            "cosT": cst["cosT"], "sinT": cst["sinT"],
            "rmat": cst["rmat"], "ident": cst["ident"],
            "masks": cst["masks"], "ksT": cst["ksT"],
            "ones_col": cst["ones_col"],
            "ones_row": np.ones((1, HD), dtype=np.float32),
        })

    res = run_bass_kernel_spmd(nc, in_maps, core_ids=list(range(NCORES)),
                               trace=_trace)
    acc = res.results[0]["out"].astype(np.float64)
    for d in range(1, NCORES):
        acc += res.results[d]["out"]
    # out_t[t, n, p, f] -> out[128t+p, 512n+f]
    outp = acc.transpose(0, 2, 1, 3).reshape(S, D).astype(np.float32)
    outp = outp.reshape(1, S, D)
    if _trace:
        _CACHE["last_results"] = res
    return outp



# revision 2
# speedup vs baseline: 1.3501x; 1.3501x over previous
"""Trainium2 Bass kernel for nn_AttnAdapter: GQA attention with RoPE,
region-based enhance/suppress score scaling, causal mask, o_proj.

Sharding: tensor-parallel over heads across 8 NeuronCores. Core d holds
q-heads 4d..4d+3 (wq rows), kv-head d (wk/wv rows), and wo columns
512d..512(d+1). Each core computes a full [S, D] partial of the output;
the host sums the 8 partials (the TP all-reduce, done at unshard time).

v2: all-bf16 matmuls (FWL weight loads), weights loaded once, fused
per-j pipeline (projection j+1 overlaps attention j), softmax denom
accumulated on DVE instead of per-tile PE matmuls, diagonal score/AV
matmuls trimmed to the causal width, region scale folded into the exp
activation's per-partition scale operand, outputs stored bf16.
"""

import math

import numpy as np

# ---- problem constants (hardcoded; kernel.py must be self-contained) ----
S = 2048          # sequence length
D = 4096          # model dim
HD = 128          # head dim
NCORES = 8
QH = 4            # q heads per core
SYS_LEN, IMG_LEN = 35, 576
BOUND = SYS_LEN + IMG_LEN          # 611
ENH, SUP = 1.5, 0.5
ROPE_BASE = 10000.0

J = 4             # sq tiles of 512
NSK = 16          # sk tiles of 128
DCH = 32          # D chunks of 128

_CACHE = {}


def _bf16(x):
    import ml_dtypes
    return np.ascontiguousarray(np.asarray(x, dtype=ml_dtypes.bfloat16))


def _host_constants():
    import ml_dtypes
    inv_freq = 1.0 / (ROPE_BASE ** (np.arange(0, HD, 2, dtype=np.float32) / HD))
    pos = np.arange(S, dtype=np.float32)
    freqs = pos[:, None] * inv_freq[None, :]              # [S, 64]
    emb = np.concatenate([freqs, freqs], axis=-1)         # [S, 128]
    cosT = _bf16(np.cos(emb).T)                           # [128, S]
    sinT = _bf16(np.sin(emb).T)

    # rotate_half as a matmul: rot = R @ q (in [hd, s] layout).
    # matmul(out, lhsT, rhs) = lhsT.T @ rhs, so feed RT = R.T.
    RT = np.zeros((HD, HD), dtype=np.float32)
    half = HD // 2
    for c in range(half):
        RT[c + half, c] = -1.0      # rot[c] = -q[c+64]
    for c in range(half, HD):
        RT[c - half, c] = 1.0       # rot[c] = q[c-64]
    rmat = _bf16(RT)

    ident = _bf16(np.eye(HD, dtype=np.float32))

    # Diagonal-tile causal mask [128, 128]: within the first 128-col block
    # of a diagonal tile, col c valid iff c >= p. Same for every delta.
    p = np.arange(128)[:, None]
    c = np.arange(128)[None, :]
    tri = _bf16((c >= p).astype(np.float32))

    # key_scale in partition layout per sk-tile: ksT[p, i] = scale(128*i+p)
    kpos = np.arange(S)
    key_scale = np.where(kpos < SYS_LEN, SUP,
                         np.where(kpos < BOUND, ENH, 1.0)).astype(np.float32)
    ksT = np.ascontiguousarray(key_scale[:5 * 128].reshape(5, 128).T)  # [128, 5]

    ones128 = np.ones((HD, HD), dtype=np.float32)
    return dict(cosT=cosT, sinT=sinT, rmat=rmat, ident=ident, tri=tri,
                ksT=ksT, ones128=ones128)


def _build_bass():
    import concourse.bass as bass
    import concourse.mybir as mybir
    from concourse.tile import TileContext
    from contextlib import ExitStack

    f32 = mybir.dt.float32
    f32r = mybir.dt.float32r
    bf16 = mybir.dt.bfloat16

    nc = bass.Bass()
    # xTt[j, d, p, f] = x.T[128d+p, 512j+f] -- each (j,d) tile contiguous
    xTt = nc.dram_tensor("xTt", [J, DCH, 128, 512], bf16, kind="ExternalInput")
    # wqkv[d, p, c]: c 0:128=wkT chunk, 128:256=wvT chunk, 256:768=wqT chunk
    wqkv = nc.dram_tensor("wqkv", [DCH, 128, 768], bf16, kind="ExternalInput")
    # woT[n, h, p, f] = woT[128h+p, 512n+f]
    woT = nc.dram_tensor("woT", [8, QH, 128, 512], bf16, kind="ExternalInput")
    cosT_d = nc.dram_tensor("cosT", [HD, S], bf16, kind="ExternalInput")
    sinT_d = nc.dram_tensor("sinT", [HD, S], bf16, kind="ExternalInput")
    rmat_d = nc.dram_tensor("rmat", [HD, HD], bf16, kind="ExternalInput")
    ident_d = nc.dram_tensor("ident", [HD, HD], bf16, kind="ExternalInput")
    tri_d = nc.dram_tensor("tri", [HD, HD], bf16, kind="ExternalInput")
    ksT_d = nc.dram_tensor("ksT", [HD, 5], f32, kind="ExternalInput")
    ones128_d = nc.dram_tensor("ones128", [HD, HD], f32r, kind="ExternalInput")
    # out[n, t, p, f] = out[128t+p, 512n+f], bf16 partial (host sums cores)
    out = nc.dram_tensor("out", [8, NSK, 128, 512], bf16, kind="ExternalOutput")

    EXP = mybir.ActivationFunctionType.Exp

    with TileContext(nc) as tc, ExitStack() as ctx:
        const = ctx.enter_context(tc.tile_pool(name="const", bufs=1))
        cosT = const.tile([HD, S], bf16)
        nc.sync.dma_start(cosT[:], cosT_d[:, :])
        sinT = const.tile([HD, S], bf16)
        nc.sync.dma_start(sinT[:], sinT_d[:, :])
        rmat = const.tile([HD, HD], bf16)
        nc.sync.dma_start(rmat[:], rmat_d[:, :])
        ident = const.tile([HD, HD], bf16)
        nc.sync.dma_start(ident[:], ident_d[:, :])
        tri = const.tile([HD, HD], bf16)
        nc.sync.dma_start(tri[:], tri_d[:, :])
        ksT = const.tile([HD, 5], f32)
        nc.sync.dma_start(ksT[:], ksT_d[:, :])
        ones128 = const.tile([HD, HD], f32r)
        nc.sync.dma_start(ones128[:], ones128_d[:, :])

        # weights: resident for the whole kernel, loaded exactly once
        wpool = ctx.enter_context(tc.tile_pool(name="wpool", bufs=1))
        wt = [wpool.tile([128, 768], bf16, name=f"wt{d}") for d in range(DCH)]
        for d in range(DCH):
            nc.sync.dma_start(wt[d][:], wqkv[d])

        persist = ctx.enter_context(tc.tile_pool(name="persist", bufs=1))
        # per-j projection outputs (bf16, post-RoPE)
        qrot = [[persist.tile([HD, 512], bf16, name=f"qrot{m}_{j}")
                 for j in range(J)] for m in range(QH)]
        krot = [persist.tile([HD, 512], bf16, name=f"krot{j}") for j in range(J)]
        vnat = [persist.tile([HD, 512], bf16, name=f"vnat{j}") for j in range(J)]
        attn = [[persist.tile([HD, 512], bf16, name=f"attn{h}_{j}")
                 for j in range(J)] for h in range(QH)]

        with tc.tile_pool(name="xp", bufs=2) as xp, \
             tc.tile_pool(name="accp", bufs=1, space="PSUM") as accp, \
             tc.tile_pool(name="tmpp", bufs=2, space="PSUM") as tmpp, \
             tc.tile_pool(name="vtp", bufs=1, space="PSUM") as vtp, \
             tc.tile_pool(name="sp", bufs=2, space="PSUM") as sp, \
             tc.tile_pool(name="avp", bufs=1, space="PSUM") as avp, \
             tc.tile_pool(name="stage", bufs=1) as stage:

            def rope(acc, dst, j):
                """acc: PSUM f32 [128,512] pre-RoPE; dst: bf16 [128,512]."""
                sq = slice(j * 512, (j + 1) * 512)
                q_sb = stage.tile([128, 512], bf16, tag="q_sb", bufs=3)
                nc.scalar.copy(q_sb[:], acc[:])
                rot_ps = tmpp.tile([128, 512], f32, tag="tmp")
                nc.tensor.matmul(rot_ps[:], rmat[:], q_sb[:],
                                 start=True, stop=True)
                t2 = stage.tile([128, 512], f32, tag="t2", bufs=2)
                nc.vector.tensor_mul(t2[:], rot_ps[:], sinT[:, sq])
                nc.vector.tensor_mul(dst[:], acc[:], cosT[:, sq])
                nc.vector.tensor_add(dst[:], dst[:], t2[:])

            for j in range(J):
                # x tiles for this j (double-buffered against j+1)
                xt = []
                for d in range(DCH):
                    t = xp.tile([128, 512], bf16, tag=f"x{d}", bufs=2)
                    nc.sync.dma_start(t[:], xTt[j, d])
                    xt.append(t)

                # ---- projections: 3 groups of 2 outputs, 2 PSUM banks ----
                # group cols in wt: (k 0:128, v 128:256), (q0,q1), (q2,q3)
                for g in range(3):
                    accA = accp.tile([128, 512], f32, tag="accA")
                    accB = accp.tile([128, 512], f32, tag="accB")
                    ca = g * 256
                    for d in range(DCH):
                        st = (d == 0)
                        sp_ = (d == DCH - 1)
                        nc.tensor.matmul(accA[:], wt[d][:, ca:ca + 128],
                                         xt[d][:], start=st, stop=sp_)
                        nc.tensor.matmul(accB[:], wt[d][:, ca + 128:ca + 256],
                                         xt[d][:], start=st, stop=sp_)
                    if g == 0:
                        # k -> rope -> krot[j]; v -> transpose -> vnat[j]
                        rope(accA, krot[j], j)
                        v_sb = stage.tile([128, 512], bf16, tag="v_sb", bufs=2)
                        nc.scalar.copy(v_sb[:], accB[:])
                        vt_ps = vtp.tile([128, 512], bf16, tag="vt")
                        for b in range(4):
                            nc.tensor.transpose(
                                vt_ps[:, b * 128:(b + 1) * 128],
                                v_sb[:, b * 128:(b + 1) * 128], ident[:])
                        nc.vector.tensor_copy(vnat[j][:], vt_ps[:])
                    else:
                        rope(accA, qrot[2 * g - 2][j], j)
                        rope(accB, qrot[2 * g - 1][j], j)

                # ---- attention for this j ----
                ni = 4 * j + 4
                for h in range(QH):
                    acc_av = avp.tile([128, 512], f32, tag="av")
                    acc_e = stage.tile([128, 512], f32r, tag="acc_e", bufs=2)
                    for i in range(ni):
                        delta = i - 4 * j
                        c0 = 128 * delta if delta > 0 else 0
                        W = 512 - c0
                        s_ps = sp.tile([128, 512], f32, tag="s")
                        nc.tensor.matmul(
                            s_ps[:, c0:512],
                            krot[i // 4][:, (i % 4) * 128:(i % 4 + 1) * 128],
                            qrot[h][j][:, c0:512], start=True, stop=True)
                        e_sb = stage.tile([128, 512], bf16, tag="e", bufs=4)
                        # region enhance/suppress folded into exp's scale
                        if i < 5 and j >= 2:
                            nc.scalar.activation(e_sb[:, c0:512],
                                                 s_ps[:, c0:512], EXP,
                                                 scale=ksT[:, i:i + 1])
                        elif i < 5 and j == 1:
                            cs = BOUND - 512   # 99: rows >= BOUND scaled
                            if c0 < cs:
                                nc.scalar.activation(e_sb[:, c0:cs],
                                                     s_ps[:, c0:cs], EXP)
                                nc.scalar.activation(e_sb[:, cs:512],
                                                     s_ps[:, cs:512], EXP,
                                                     scale=ksT[:, i:i + 1])
                            else:
                                nc.scalar.activation(e_sb[:, c0:512],
                                                     s_ps[:, c0:512], EXP,
                                                     scale=ksT[:, i:i + 1])
                        else:
                            nc.scalar.activation(e_sb[:, c0:512],
                                                 s_ps[:, c0:512], EXP)
                        if delta >= 0:
                            # causal mask on the partial 128-col block
                            nc.vector.tensor_mul(e_sb[:, c0:c0 + 128],
                                                 e_sb[:, c0:c0 + 128], tri[:])
                        # denominator accumulation on DVE
                        if i == 0:
                            nc.vector.tensor_copy(acc_e[:], e_sb[:])
                        else:
                            nc.vector.tensor_add(acc_e[:, c0:512],
                                                 acc_e[:, c0:512],
                                                 e_sb[:, c0:512])
                        nc.tensor.matmul(
                            acc_av[:, c0:512],
                            vnat[i // 4][:, (i % 4) * 128:(i % 4 + 1) * 128],
                            e_sb[:, c0:512], start=(i == 0), stop=(i == ni - 1),
                            skip_group_check=True)
                    # denom replicated to all partitions via ones matmul,
                    # then reciprocal + normalize on DVE
                    dn_ps = tmpp.tile([128, 512], f32, tag="tmp")
                    nc.tensor.matmul(dn_ps[:], ones128[:], acc_e[:],
                                     start=True, stop=True)
                    rec = stage.tile([128, 512], f32, tag="rec", bufs=2)
                    nc.vector.reciprocal(rec[:], dn_ps[:])
                    nc.vector.tensor_mul(attn[h][j][:], acc_av[:], rec[:])

        # ---------------- o_proj ---------------------------------
        with tc.tile_pool(name="wop", bufs=1) as wop, \
             tc.tile_pool(name="op", bufs=2, space="PSUM") as op, \
             tc.tile_pool(name="ost", bufs=1) as ost:
            for n in range(8):
                wo_t = []
                for h in range(QH):
                    t = wop.tile([128, 512], bf16, tag=f"wo{h}", bufs=2)
                    nc.sync.dma_start(t[:], woT[n, h])
                    wo_t.append(t)
                for t_ in range(NSK):
                    o_ps = op.tile([128, 512], f32, tag="o")
                    for h in range(QH):
                        nc.tensor.matmul(
                            o_ps[:],
                            attn[h][t_ // 4][:, (t_ % 4) * 128:(t_ % 4 + 1) * 128],
                            wo_t[h][:], start=(h == 0), stop=(h == QH - 1))
                    o_sb = ost.tile([128, 512], bf16, tag="o_sb", bufs=4)
                    nc.scalar.copy(o_sb[:], o_ps[:])
                    nc.sync.dma_start(out[n, t_], o_sb[:])

    # Split multi-wait instructions onto standalone EventSemaphore
    # instructions.
    import bass_rust
    bass_rust.generate_event_semaphores(nc)
    return nc


def _get_compiled():
    if "nc" not in _CACHE:
        _CACHE["nc"] = _build_bass()
        _CACHE["const"] = _host_constants()
    return _CACHE["nc"], _CACHE["const"]


def kernel(hidden_states, wq, wk, wv, wo, _trace=False):
    from concourse.bass_utils import run_bass_kernel_spmd

    nc, cst = _get_compiled()

    x = np.asarray(hidden_states, dtype=np.float32).reshape(S, D)
    xT = np.ascontiguousarray(x.T)                       # [D, S]
    xTt = _bf16(xT.reshape(DCH, 128, J, 512).transpose(2, 0, 1, 3))
    wq = np.asarray(wq, dtype=np.float32)
    wk = np.asarray(wk, dtype=np.float32)
    wv = np.asarray(wv, dtype=np.float32)
    wo = np.asarray(wo, dtype=np.float32)
    scale = 1.0 / math.sqrt(HD)

    in_maps = []
    for d in range(NCORES):
        wq_d = (wq[d * QH * HD:(d + 1) * QH * HD] * scale).T  # [D, 512]
        wk_d = wk[d * HD:(d + 1) * HD].T                      # [D, 128]
        wv_d = wv[d * HD:(d + 1) * HD].T                      # [D, 128]
        wqkv_d = np.concatenate(
            [wk_d, wv_d, wq_d], axis=1).reshape(DCH, 128, 768)
        wo_d = wo[:, d * QH * HD:(d + 1) * QH * HD].T         # [512, D]
        woT_d = np.ascontiguousarray(
            wo_d.reshape(QH, 128, 8, 512).transpose(2, 0, 1, 3))
        in_maps.append({
            "xTt": xTt,
            "wqkv": _bf16(wqkv_d),
            "woT": _bf16(woT_d),
            "cosT": cst["cosT"], "sinT": cst["sinT"],
            "rmat": cst["rmat"], "ident": cst["ident"],
            "tri": cst["tri"], "ksT": cst["ksT"],
            "ones128": cst["ones128"],
        })

    res = run_bass_kernel_spmd(nc, in_maps, core_ids=list(range(NCORES)),
                               trace=_trace)
    acc = res.results[0]["out"].astype(np.float64)
    for d in range(1, NCORES):
        acc += res.results[d]["out"].astype(np.float64)
    # out[n, t, p, f] -> out[128t+p, 512n+f]
    outp = acc.transpose(1, 2, 0, 3).reshape(S, D).astype(np.float32)
    outp = outp.reshape(1, S, D)
    if _trace:
        _CACHE["last_results"] = res
    return outp


# revision 4
# speedup vs baseline: 1.5332x; 1.1357x over previous
"""Trainium2 Bass kernel for nn_AttnAdapter: GQA attention with RoPE,
region-based enhance/suppress score scaling, causal mask, o_proj.

Sharding: tensor-parallel over heads across 8 NeuronCores. Core d holds
q-heads 4d..4d+3 (wq rows), kv-head d (wk/wv rows), and wo columns
512d..512(d+1). Each core computes a full [S, D] partial of the output;
the host sums the 8 partials (the TP all-reduce, done at unshard time).

v3: all-bf16 matmuls (FWL weight loads), weights loaded once, software-
pipelined emission: projection matmuls for seq-tile j+1 are interleaved
into the attention stream for seq-tile j (and the first o_proj tiles
into the last attention tile) so the PE never stalls on the exp stream.
Softmax denom is accumulated on DVE in bf16, replicated across
partitions with a ones-matmul, and inverted with ACT Ln/Exp (the DVE
reciprocal costs 3.4us/tile). Outputs are stored bf16 and summed on
host.
"""

import math

import numpy as np

# ---- problem constants (hardcoded; kernel.py must be self-contained) ----
S = 2048          # sequence length
D = 4096          # model dim
HD = 128          # head dim
NCORES = 8
QH = 4            # q heads per core
SYS_LEN, IMG_LEN = 35, 576
BOUND = SYS_LEN + IMG_LEN          # 611
ENH, SUP = 1.5, 0.5
ROPE_BASE = 10000.0

J = 4             # sq tiles of 512
NSK = 16          # sk tiles of 128
DCH = 32          # D chunks of 128

_CACHE = {}


def _bf16(x):
    import ml_dtypes
    return np.ascontiguousarray(np.asarray(x, dtype=ml_dtypes.bfloat16))


def _host_constants():
    inv_freq = 1.0 / (ROPE_BASE ** (np.arange(0, HD, 2, dtype=np.float32) / HD))
    pos = np.arange(S, dtype=np.float32)
    freqs = pos[:, None] * inv_freq[None, :]              # [S, 64]
    emb = np.concatenate([freqs, freqs], axis=-1)         # [S, 128]
    cosT = _bf16(np.cos(emb).T)                           # [128, S]
    sinT = _bf16(np.sin(emb).T)

    # rotate_half as a matmul: rot = R @ q (in [hd, s] layout).
    # matmul(out, lhsT, rhs) = lhsT.T @ rhs, so feed RT = R.T.
    RT = np.zeros((HD, HD), dtype=np.float32)
    half = HD // 2
    for c in range(half):
        RT[c + half, c] = -1.0      # rot[c] = -q[c+64]
    for c in range(half, HD):
        RT[c - half, c] = 1.0       # rot[c] = q[c-64]
    rmat = _bf16(RT)

    ident = np.eye(HD, dtype=np.float32)

    # Diagonal-tile causal mask [128, 128]: within the first 128-col block
    # of a diagonal tile, col c valid iff c >= p. Same for every delta.
    p = np.arange(128)[:, None]
    c = np.arange(128)[None, :]
    tri = _bf16((c >= p).astype(np.float32))

    # key_scale in partition layout per sk-tile: ksT[p, i] = scale(128*i+p)
    kpos = np.arange(S)
    key_scale = np.where(kpos < SYS_LEN, SUP,
                         np.where(kpos < BOUND, ENH, 1.0)).astype(np.float32)
    ksT = np.ascontiguousarray(key_scale[:5 * 128].reshape(5, 128).T)  # [128, 5]

    ones128 = _bf16(np.ones((HD, HD), dtype=np.float32))
    return dict(cosT=cosT, sinT=sinT, rmat=rmat, ident=ident, tri=tri,
                ksT=ksT, ones128=ones128)


def _interleave(main, fill):
    """Merge two unit lists, spreading `fill` evenly across `main`."""
    units = []
    nf = len(fill)
    nm = max(1, len(main))
    k = 0
    for m, u in enumerate(main):
        units.append(u)
        want = (m + 1) * nf // nm
        while k < want:
            units.append(fill[k])
            k += 1
    units.extend(fill[k:])
    return units


def _build_bass():
    import concourse.bass as bass
    import concourse.mybir as mybir
    from concourse.tile import TileContext
    from contextlib import ExitStack

    f32 = mybir.dt.float32
    bf16 = mybir.dt.bfloat16

    nc = bass.Bass()
    # xTt[j, d, p, f] = x.T[128d+p, 512j+f] -- each (j,d) tile contiguous
    xTt = nc.dram_tensor("xTt", [J, DCH, 128, 512], bf16, kind="ExternalInput")
    # wqkv[d, p, c]: c 0:128=wkT chunk, 128:256=wvT chunk, 256:768=wqT chunk
    wqkv = nc.dram_tensor("wqkv", [DCH, 128, 768], bf16, kind="ExternalInput")
    # woT[n, h, p, f] = woT[128h+p, 512n+f]
    woT = nc.dram_tensor("woT", [8, QH, 128, 512], bf16, kind="ExternalInput")
    cosT_d = nc.dram_tensor("cosT", [HD, S], bf16, kind="ExternalInput")
    sinT_d = nc.dram_tensor("sinT", [HD, S], bf16, kind="ExternalInput")
    rmat_d = nc.dram_tensor("rmat", [HD, HD], bf16, kind="ExternalInput")
    ident_d = nc.dram_tensor("ident", [HD, HD], f32, kind="ExternalInput")
    tri_d = nc.dram_tensor("tri", [HD, HD], bf16, kind="ExternalInput")
    ksT_d = nc.dram_tensor("ksT", [HD, 5], f32, kind="ExternalInput")
    ones128_d = nc.dram_tensor("ones128", [HD, HD], bf16, kind="ExternalInput")
    # out[n, t, p, f] = out[128t+p, 512n+f], bf16 partial (host sums cores)
    out = nc.dram_tensor("out", [8, NSK, 128, 512], bf16, kind="ExternalOutput")

    EXP = mybir.ActivationFunctionType.Exp
    LN = mybir.ActivationFunctionType.Ln

    with TileContext(nc) as tc, ExitStack() as ctx:
        const = ctx.enter_context(tc.tile_pool(name="const", bufs=1))
        wpool = ctx.enter_context(tc.tile_pool(name="wpool", bufs=1))
        persist = ctx.enter_context(tc.tile_pool(name="persist", bufs=1))

        wt = [wpool.tile([128, 768], bf16, name=f"wt{d}") for d in range(DCH)]
        qrot = [[persist.tile([HD, 512], bf16, name=f"qrot{m}_{j}")
                 for j in range(J)] for m in range(QH)]
        krot = [persist.tile([HD, 512], bf16, name=f"krot{j}") for j in range(J)]
        vnat = [persist.tile([HD, 512], bf16, name=f"vnat{j}") for j in range(J)]
        attn = [[persist.tile([HD, 512], bf16, name=f"attn{h}_{j}")
                 for j in range(J)] for h in range(QH)]

        cosT = const.tile([HD, S], bf16)
        sinT = const.tile([HD, S], bf16)
        rmat = const.tile([HD, HD], bf16)
        ident = const.tile([HD, HD], f32)
        tri = const.tile([HD, HD], bf16)
        ksT = const.tile([HD, 5], f32)
        ones128 = const.tile([HD, HD], bf16)

        with tc.tile_pool(name="xp", bufs=2) as xp, \
             tc.tile_pool(name="wop", bufs=1) as wop, \
             tc.tile_pool(name="accp", bufs=1, space="PSUM") as accp, \
             tc.tile_pool(name="tmpp", bufs=2, space="PSUM") as tmpp, \
             tc.tile_pool(name="sp", bufs=2, space="PSUM") as sp, \
             tc.tile_pool(name="avp", bufs=2, space="PSUM") as avp, \
             tc.tile_pool(name="stage", bufs=1) as stage:

            xt = [[None] * DCH for _ in range(J)]

            def rope_unit(acc, dst, j):
                def emit():
                    sq = slice(j * 512, (j + 1) * 512)
                    q_sb = stage.tile([128, 512], bf16, tag="q_sb", bufs=3)
                    nc.scalar.copy(q_sb[:], acc[:])
                    rot_ps = tmpp.tile([128, 512], f32, tag="tmp")
                    nc.tensor.matmul(rot_ps[:], rmat[:], q_sb[:],
                                     start=True, stop=True)
                    t2 = stage.tile([128, 512], f32, tag="t2", bufs=2)
                    nc.vector.tensor_mul(t2[:], rot_ps[:], sinT[:, sq])
                    nc.vector.tensor_mul(dst[:], acc[:], cosT[:, sq])
                    nc.vector.tensor_add(dst[:], dst[:], t2[:])
                return emit

            def a_units(j, with_consts=False, with_x=True):
                """Projection+RoPE emission units for seq-tile j."""
                units = []
                accs = {}

                def dma_unit(d):
                    def emit():
                        if j == 0:
                            nc.sync.dma_start(wt[d][:], wqkv[d])
                        if with_x:
                            t = xp.tile([128, 512], bf16, tag=f"x{d}", bufs=2, name=f"x{d}")
                            nc.sync.dma_start(t[:], xTt[j, d])
                            xt[j][d] = t
                    return emit

                def mm_unit(g, d):
                    def emit():
                        if d == 0:
                            accs[g] = (accp.tile([128, 512], f32, tag="accA", name="accA"),
                                       accp.tile([128, 512], f32, tag="accB", name="accB"))
                        accA, accB = accs[g]
                        ca = g * 256
                        st = (d == 0)
                        sp_ = (d == DCH - 1)
                        nc.tensor.matmul(accA[:], wt[d][:, ca:ca + 128],
                                         xt[j][d][:], start=st, stop=sp_)
                        nc.tensor.matmul(accB[:], wt[d][:, ca + 128:ca + 256],
                                         xt[j][d][:], start=st, stop=sp_)
                    return emit

                def vt_unit(g):
                    def emit():
                        accB = accs[g][1]
                        v_sb = stage.tile([128, 512], f32, tag="v_sb", bufs=2)
                        nc.scalar.copy(v_sb[:], accB[:])
                        vt_ps = tmpp.tile([128, 512], f32, tag="tmp")
                        for b in range(4):
                            nc.tensor.transpose(
                                vt_ps[:, b * 128:(b + 1) * 128],
                                v_sb[:, b * 128:(b + 1) * 128], ident[:])
                        nc.vector.tensor_copy(vnat[j][:], vt_ps[:])
                    return emit

                for d in range(DCH):
                    units.append(dma_unit(d))
                if with_consts:
                    def cdma():
                        nc.sync.dma_start(cosT[:], cosT_d[:, :])
                        nc.sync.dma_start(sinT[:], sinT_d[:, :])
                        nc.sync.dma_start(rmat[:], rmat_d[:, :])
                        nc.sync.dma_start(ident[:], ident_d[:, :])
                        nc.sync.dma_start(tri[:], tri_d[:, :])
                        nc.sync.dma_start(ksT[:], ksT_d[:, :])
                        nc.sync.dma_start(ones128[:], ones128_d[:, :])
                    units.append(cdma)
                # group 0: (k, v); group 1: (q0, q1); group 2: (q2, q3)
                for d in range(DCH):
                    units.append(mm_unit(0, d))
                units.append(rope_unit(None, krot[j], j))  # acc patched below
                units.append(vt_unit(0))
                for d in range(DCH):
                    units.append(mm_unit(1, d))
                units.append(rope_unit(None, qrot[0][j], j))
                units.append(rope_unit(None, qrot[1][j], j))
                for d in range(DCH):
                    units.append(mm_unit(2, d))
                units.append(rope_unit(None, qrot[2][j], j))
                units.append(rope_unit(None, qrot[3][j], j))

                # fix rope units to read the right acc lazily
                def make_rope(g, which, dst):
                    def emit():
                        rope_unit(accs[g][which], dst, j)()
                    return emit
                units[DCH + (1 if with_consts else 0) + DCH] = \
                    make_rope(0, 0, krot[j])
                base = DCH + (1 if with_consts else 0)
                units[base + 2 * DCH + 2] = make_rope(1, 0, qrot[0][j])
                units[base + 2 * DCH + 3] = make_rope(1, 1, qrot[1][j])
                units[base + 3 * DCH + 4] = make_rope(2, 0, qrot[2][j])
                units[base + 3 * DCH + 5] = make_rope(2, 1, qrot[3][j])
                return units

            def b_units(j):
                """Attention emission units for seq-tile j."""
                units = []
                ni = 4 * j + 4
                state = {}

                def tile_unit(h, i):
                    def emit():
                        if i == 0:
                            state["av"] = avp.tile([128, 512], f32, tag="av", name="av")
                            state["acc_e"] = stage.tile([128, 512], bf16,
                                                        tag="acc_e", bufs=2, name="acc_e")
                        acc_av = state["av"]
                        acc_e = state["acc_e"]
                        delta = i - 4 * j
                        c0 = 128 * delta if delta > 0 else 0
                        s_ps = sp.tile([128, 512], f32, tag="s")
                        nc.tensor.matmul(
                            s_ps[:, c0:512],
                            krot[i // 4][:, (i % 4) * 128:(i % 4 + 1) * 128],
                            qrot[h][j][:, c0:512], start=True, stop=True)
                        e_sb = stage.tile([128, 512], bf16, tag="e", bufs=4)
                        # region enhance/suppress folded into exp's scale
                        if i < 5 and j >= 2:
                            nc.scalar.activation(e_sb[:, c0:512],
                                                 s_ps[:, c0:512], EXP,
                                                 scale=ksT[:, i:i + 1])
                        elif i < 5 and j == 1:
                            cs = BOUND - 512   # 99: rows >= BOUND scaled
                            nc.scalar.activation(e_sb[:, c0:cs],
                                                 s_ps[:, c0:cs], EXP)
                            nc.scalar.activation(e_sb[:, cs:512],
                                                 s_ps[:, cs:512], EXP,
                                                 scale=ksT[:, i:i + 1])
                        else:
                            nc.scalar.activation(e_sb[:, c0:512],
                                                 s_ps[:, c0:512], EXP)
                        if delta >= 0:
                            # causal mask on the partial 128-col block
                            nc.vector.tensor_mul(e_sb[:, c0:c0 + 128],
                                                 e_sb[:, c0:c0 + 128], tri[:])
                        # denominator accumulation on DVE (bf16, 2x rate)
                        if i == 0:
                            nc.vector.tensor_copy(acc_e[:], e_sb[:])
                        else:
                            nc.vector.tensor_add(acc_e[:, c0:512],
                                                 acc_e[:, c0:512],
                                                 e_sb[:, c0:512])
                        nc.tensor.matmul(
                            acc_av[:, c0:512],
                            vnat[i // 4][:, (i % 4) * 128:(i % 4 + 1) * 128],
                            e_sb[:, c0:512], start=(i == 0), stop=(i == ni - 1),
                            skip_group_check=True)
                    return emit

                def fin_unit(h):
                    def emit():
                        acc_av = state["av"]
                        acc_e = state["acc_e"]
                        # denom replicated to all partitions via ones matmul;
                        # 1/x = exp(-ln(x)) on ACT (DVE reciprocal is 3.4us)
                        dn_ps = tmpp.tile([128, 512], f32, tag="tmp")
                        nc.tensor.matmul(dn_ps[:], ones128[:], acc_e[:],
                                         start=True, stop=True)
                        lrec = stage.tile([128, 512], f32, tag="lrec", bufs=2)
                        nc.scalar.activation(lrec[:], dn_ps[:], LN)
                        rec = stage.tile([128, 512], f32, tag="rec", bufs=2)
                        nc.scalar.activation(rec[:], lrec[:], EXP, scale=-1.0)
                        nc.vector.tensor_mul(attn[h][j][:], acc_av[:], rec[:])
                    return emit

                for h in range(QH):
                    for i in range(ni):
                        units.append(tile_unit(h, i))
                    units.append(fin_unit(h))
                return units

            def c_units(n_range, t_range, par):
                """o_proj emission units; o_ps borrows the idle accp banks."""
                units = []
                wo_t = {}

                def wo_dma(n):
                    def emit():
                        tiles = []
                        for h in range(QH):
                            t = wop.tile([128, 512], bf16, tag=f"wo{h}", bufs=2, name=f"wo{h}")
                            nc.sync.dma_start(t[:], woT[n, h])
                            tiles.append(t)
                        wo_t[n] = tiles
                    return emit

                def ct_unit(n, t_, k):
                    def emit():
                        tag = "accA" if k % 2 == 0 else "accB"
                        o_ps = accp.tile([128, 512], f32, tag=tag)
                        for h in range(QH):
                            nc.tensor.matmul(
                                o_ps[:],
                                attn[h][t_ // 4][:,
                                                 (t_ % 4) * 128:(t_ % 4 + 1) * 128],
                                wo_t[n][h][:], start=(h == 0), stop=(h == QH - 1))
                        o_sb = stage.tile([128, 512], bf16, tag="o_sb", bufs=4)
                        if k % 2 == 0:
                            nc.scalar.copy(o_sb[:], o_ps[:])
                        else:
                            nc.vector.tensor_copy(o_sb[:], o_ps[:])
                        nc.sync.dma_start(out[n, t_], o_sb[:])
                    return emit

                k = par
                for n in n_range:
                    units.append(wo_dma(n))
                    for t_ in t_range:
                        units.append(ct_unit(n, t_, k))
                        k += 1
                return units

            # ---- emission schedule: A(0), then B(j) || A(j+1), B(3) || C ----
            for u in a_units(0, with_consts=True):
                u()
            for j in range(J):
                main = b_units(j)
                if j + 1 < J:
                    fill = a_units(j + 1)
                elif j == J - 1:
                    fill = c_units(range(8), range(12), 0)
                for u in _interleave(main, fill):
                    u()
            # o_proj tail: t 12..15 for all n (wo re-streamed)
            for u in c_units(range(8), range(12, 16), 0):
                u()

    # Split multi-wait instructions onto standalone EventSemaphore
    # instructions.
    import bass_rust
    bass_rust.generate_event_semaphores(nc)
    return nc


def _get_compiled():
    if "nc" not in _CACHE:
        _CACHE["nc"] = _build_bass()
        _CACHE["const"] = _host_constants()
    return _CACHE["nc"], _CACHE["const"]


def kernel(hidden_states, wq, wk, wv, wo, _trace=False):
    from concourse.bass_utils import run_bass_kernel_spmd

    nc, cst = _get_compiled()

    x = np.asarray(hidden_states, dtype=np.float32).reshape(S, D)
    xT = np.ascontiguousarray(x.T)                       # [D, S]
    xTt = _bf16(xT.reshape(DCH, 128, J, 512).transpose(2, 0, 1, 3))
    wq = np.asarray(wq, dtype=np.float32)
    wk = np.asarray(wk, dtype=np.float32)
    wv = np.asarray(wv, dtype=np.float32)
    wo = np.asarray(wo, dtype=np.float32)
    scale = 1.0 / math.sqrt(HD)

    in_maps = []
    for d in range(NCORES):
        wq_d = (wq[d * QH * HD:(d + 1) * QH * HD] * scale).T  # [D, 512]
        wk_d = wk[d * HD:(d + 1) * HD].T                      # [D, 128]
        wv_d = wv[d * HD:(d + 1) * HD].T                      # [D, 128]
        wqkv_d = np.concatenate(
            [wk_d, wv_d, wq_d], axis=1).reshape(DCH, 128, 768)
        wo_d = wo[:, d * QH * HD:(d + 1) * QH * HD].T         # [512, D]
        woT_d = np.ascontiguousarray(
            wo_d.reshape(QH, 128, 8, 512).transpose(2, 0, 1, 3))
        in_maps.append({
            "xTt": xTt,
            "wqkv": _bf16(wqkv_d),
            "woT": _bf16(woT_d),
            "cosT": cst["cosT"], "sinT": cst["sinT"],
            "rmat": cst["rmat"], "ident": cst["ident"],
            "tri": cst["tri"], "ksT": cst["ksT"],
            "ones128": cst["ones128"],
        })

    res = run_bass_kernel_spmd(nc, in_maps, core_ids=list(range(NCORES)),
                               trace=_trace)
    acc = res.results[0]["out"].astype(np.float64)
    for d in range(1, NCORES):
        acc += res.results[d]["out"].astype(np.float64)
    # out[n, t, p, f] -> out[128t+p, 512n+f]
    outp = acc.transpose(1, 2, 0, 3).reshape(S, D).astype(np.float32)
    outp = outp.reshape(1, S, D)
    if _trace:
        _CACHE["last_results"] = res
    return outp


# revision 5
# speedup vs baseline: 1.5478x; 1.0095x over previous
"""Trainium2 Bass kernel for nn_AttnAdapter: GQA attention with RoPE,
region-based enhance/suppress score scaling, causal mask, o_proj.

Sharding: tensor-parallel over heads across 8 NeuronCores. Core d holds
q-heads 4d..4d+3 (wq rows), kv-head d (wk/wv rows), and wo columns
512d..512(d+1). Each core computes a full [S, D] partial of the output;
the host sums the 8 partials (the TP all-reduce, done at unshard time).

v3: all-bf16 matmuls (FWL weight loads), weights loaded once, software-
pipelined emission: projection matmuls for seq-tile j+1 are interleaved
into the attention stream for seq-tile j (and the first o_proj tiles
into the last attention tile) so the PE never stalls on the exp stream.
Softmax denom is accumulated on DVE in bf16, replicated across
partitions with a ones-matmul, and inverted with ACT Ln/Exp (the DVE
reciprocal costs 3.4us/tile). Outputs are stored bf16 and summed on
host.
"""

import math

import numpy as np

# ---- problem constants (hardcoded; kernel.py must be self-contained) ----
S = 2048          # sequence length
D = 4096          # model dim
HD = 128          # head dim
NCORES = 8
QH = 4            # q heads per core
SYS_LEN, IMG_LEN = 35, 576
BOUND = SYS_LEN + IMG_LEN          # 611
ENH, SUP = 1.5, 0.5
ROPE_BASE = 10000.0

J = 4             # sq tiles of 512
NSK = 16          # sk tiles of 128
DCH = 32          # D chunks of 128

_CACHE = {}


def _bf16(x):
    import ml_dtypes
    return np.ascontiguousarray(np.asarray(x, dtype=ml_dtypes.bfloat16))


def _host_constants():
    inv_freq = 1.0 / (ROPE_BASE ** (np.arange(0, HD, 2, dtype=np.float32) / HD))
    pos = np.arange(S, dtype=np.float32)
    freqs = pos[:, None] * inv_freq[None, :]              # [S, 64]
    emb = np.concatenate([freqs, freqs], axis=-1)         # [S, 128]
    cosT = _bf16(np.cos(emb).T)                           # [128, S]
    sinT = _bf16(np.sin(emb).T)

    # rotate_half on DVE via partition-offset reads:
    # t2[0:64] = -q[64:128]*sin[0:64]; t2[64:128] = q[0:64]*sin[64:128].
    sinN = np.concatenate([-np.sin(emb).T[:HD // 2], np.sin(emb).T[HD // 2:]])
    sinN = _bf16(sinN)

    ident = np.eye(HD, dtype=np.float32)

    # Diagonal-tile causal mask [128, 128]: within the first 128-col block
    # of a diagonal tile, col c valid iff c >= p. Same for every delta.
    p = np.arange(128)[:, None]
    c = np.arange(128)[None, :]
    tri = _bf16((c >= p).astype(np.float32))

    # key_scale in partition layout per sk-tile: ksT[p, i] = scale(128*i+p)
    kpos = np.arange(S)
    key_scale = np.where(kpos < SYS_LEN, SUP,
                         np.where(kpos < BOUND, ENH, 1.0)).astype(np.float32)
    ksT = np.ascontiguousarray(key_scale[:5 * 128].reshape(5, 128).T)  # [128, 5]

    ones128 = _bf16(np.ones((HD, HD), dtype=np.float32))
    return dict(cosT=cosT, sinT=sinT, sinN=sinN, ident=ident, tri=tri,
                ksT=ksT, ones128=ones128)


def _interleave(main, fill):
    """Merge two unit lists, spreading `fill` evenly across `main`."""
    units = []
    nf = len(fill)
    nm = max(1, len(main))
    k = 0
    for m, u in enumerate(main):
        units.append(u)
        want = (m + 1) * nf // nm
        while k < want:
            units.append(fill[k])
            k += 1
    units.extend(fill[k:])
    return units


def _build_bass():
    import concourse.bass as bass
    import concourse.mybir as mybir
    from concourse.tile import TileContext
    from contextlib import ExitStack

    f32 = mybir.dt.float32
    bf16 = mybir.dt.bfloat16

    nc = bass.Bass()
    # xTt[j, d, p, f] = x.T[128d+p, 512j+f] -- each (j,d) tile contiguous
    xTt = nc.dram_tensor("xTt", [J, DCH, 128, 512], bf16, kind="ExternalInput")
    # wqkv[d, p, c]: c 0:128=wkT chunk, 128:256=wvT chunk, 256:768=wqT chunk
    wqkv = nc.dram_tensor("wqkv", [DCH, 128, 768], bf16, kind="ExternalInput")
    # woT[n, h, p, f] = woT[128h+p, 512n+f]
    woT = nc.dram_tensor("woT", [8, QH, 128, 512], bf16, kind="ExternalInput")
    cosT_d = nc.dram_tensor("cosT", [HD, S], bf16, kind="ExternalInput")
    sinT_d = nc.dram_tensor("sinT", [HD, S], bf16, kind="ExternalInput")
    sinN_d = nc.dram_tensor("sinN", [HD, S], bf16, kind="ExternalInput")
    ident_d = nc.dram_tensor("ident", [HD, HD], f32, kind="ExternalInput")
    tri_d = nc.dram_tensor("tri", [HD, HD], bf16, kind="ExternalInput")
    ksT_d = nc.dram_tensor("ksT", [HD, 5], f32, kind="ExternalInput")
    ones128_d = nc.dram_tensor("ones128", [HD, HD], bf16, kind="ExternalInput")
    # out[n, t, p, f] = out[128t+p, 512n+f], bf16 partial (host sums cores)
    out = nc.dram_tensor("out", [8, NSK, 128, 512], bf16, kind="ExternalOutput")

    EXP = mybir.ActivationFunctionType.Exp
    LN = mybir.ActivationFunctionType.Ln

    with TileContext(nc) as tc, ExitStack() as ctx:
        const = ctx.enter_context(tc.tile_pool(name="const", bufs=1))
        wpool = ctx.enter_context(tc.tile_pool(name="wpool", bufs=1))
        persist = ctx.enter_context(tc.tile_pool(name="persist", bufs=1))

        wt = [wpool.tile([128, 768], bf16, name=f"wt{d}") for d in range(DCH)]
        qrot = [[persist.tile([HD, 512], bf16, name=f"qrot{m}_{j}")
                 for j in range(J)] for m in range(QH)]
        krot = [persist.tile([HD, 512], bf16, name=f"krot{j}") for j in range(J)]
        vnat = [persist.tile([HD, 512], bf16, name=f"vnat{j}") for j in range(J)]
        attn = [[persist.tile([HD, 512], bf16, name=f"attn{h}_{j}")
                 for j in range(J)] for h in range(QH)]

        cosT = const.tile([HD, S], bf16)
        sinT = const.tile([HD, S], bf16)
        sinN = const.tile([HD, S], bf16)
        ident = const.tile([HD, HD], f32)
        tri = const.tile([HD, HD], bf16)
        ksT = const.tile([HD, 5], f32)
        ones128 = const.tile([HD, HD], bf16)

        with tc.tile_pool(name="xp", bufs=2) as xp, \
             tc.tile_pool(name="wop", bufs=1) as wop, \
             tc.tile_pool(name="accp", bufs=1, space="PSUM") as accp, \
             tc.tile_pool(name="tmpp", bufs=1, space="PSUM") as tmpp, \
             tc.tile_pool(name="sp", bufs=3, space="PSUM") as sp, \
             tc.tile_pool(name="avp", bufs=2, space="PSUM") as avp, \
             tc.tile_pool(name="stage", bufs=1) as stage:

            xt = [[None] * DCH for _ in range(J)]

            def rope_unit(acc, dst, j):
                def emit():
                    sq = slice(j * 512, (j + 1) * 512)
                    t2 = stage.tile([128, 512], f32, tag="t2", bufs=2)
                    nc.vector.tensor_mul(t2[0:64, :], acc[64:128, :],
                                         sinN[0:64, sq])
                    nc.vector.tensor_mul(t2[64:128, :], acc[0:64, :],
                                         sinN[64:128, sq])
                    nc.vector.tensor_mul(dst[:], acc[:], cosT[:, sq])
                    nc.vector.tensor_add(dst[:], dst[:], t2[:])
                return emit

            def a_units(j, with_consts=False, with_x=True):
                """Projection+RoPE emission units for seq-tile j."""
                units = []
                accs = {}

                def dma_unit(d):
                    def emit():
                        if j == 0:
                            nc.sync.dma_start(wt[d][:], wqkv[d])
                        if with_x:
                            t = xp.tile([128, 512], bf16, tag=f"x{d}", bufs=2, name=f"x{d}")
                            nc.sync.dma_start(t[:], xTt[j, d])
                            xt[j][d] = t
                    return emit

                def mm_unit(g, d):
                    def emit():
                        if d == 0:
                            accs[g] = (accp.tile([128, 512], f32, tag="accA", name="accA"),
                                       accp.tile([128, 512], f32, tag="accB", name="accB"))
                        accA, accB = accs[g]
                        ca = g * 256
                        st = (d == 0)
                        sp_ = (d == DCH - 1)
                        nc.tensor.matmul(accA[:], wt[d][:, ca:ca + 128],
                                         xt[j][d][:], start=st, stop=sp_)
                        nc.tensor.matmul(accB[:], wt[d][:, ca + 128:ca + 256],
                                         xt[j][d][:], start=st, stop=sp_)
                    return emit

                def vt_unit(g):
                    def emit():
                        accB = accs[g][1]
                        v_sb = stage.tile([128, 512], f32, tag="v_sb", bufs=2)
                        nc.scalar.copy(v_sb[:], accB[:])
                        vt_ps = tmpp.tile([128, 512], f32, tag="tmp")
                        for b in range(4):
                            nc.tensor.transpose(
                                vt_ps[:, b * 128:(b + 1) * 128],
                                v_sb[:, b * 128:(b + 1) * 128], ident[:])
                        nc.vector.tensor_copy(vnat[j][:], vt_ps[:])
                    return emit

                for d in range(DCH):
                    units.append(dma_unit(d))
                if with_consts:
                    def cdma():
                        nc.sync.dma_start(cosT[:], cosT_d[:, :])
                        nc.sync.dma_start(sinT[:], sinT_d[:, :])
                        nc.sync.dma_start(sinN[:], sinN_d[:, :])
                        nc.sync.dma_start(ident[:], ident_d[:, :])
                        nc.sync.dma_start(tri[:], tri_d[:, :])
                        nc.sync.dma_start(ksT[:], ksT_d[:, :])
                        nc.sync.dma_start(ones128[:], ones128_d[:, :])
                    units.append(cdma)
                # group 0: (k, v); group 1: (q0, q1); group 2: (q2, q3)
                for d in range(DCH):
                    units.append(mm_unit(0, d))
                units.append(rope_unit(None, krot[j], j))  # acc patched below
                units.append(vt_unit(0))
                for d in range(DCH):
                    units.append(mm_unit(1, d))
                units.append(rope_unit(None, qrot[0][j], j))
                units.append(rope_unit(None, qrot[1][j], j))
                for d in range(DCH):
                    units.append(mm_unit(2, d))
                units.append(rope_unit(None, qrot[2][j], j))
                units.append(rope_unit(None, qrot[3][j], j))

                # fix rope units to read the right acc lazily
                def make_rope(g, which, dst):
                    def emit():
                        rope_unit(accs[g][which], dst, j)()
                    return emit
                units[DCH + (1 if with_consts else 0) + DCH] = \
                    make_rope(0, 0, krot[j])
                base = DCH + (1 if with_consts else 0)
                units[base + 2 * DCH + 2] = make_rope(1, 0, qrot[0][j])
                units[base + 2 * DCH + 3] = make_rope(1, 1, qrot[1][j])
                units[base + 3 * DCH + 4] = make_rope(2, 0, qrot[2][j])
                units[base + 3 * DCH + 5] = make_rope(2, 1, qrot[3][j])
                return units

            def b_units(j):
                """Attention emission units for seq-tile j."""
                units = []
                ni = 4 * j + 4
                state = {}

                def av_mm(h, i):
                    delta = i - 4 * j
                    c0 = 128 * delta if delta > 0 else 0
                    nc.tensor.matmul(
                        state["av"][:, c0:512],
                        vnat[i // 4][:, (i % 4) * 128:(i % 4 + 1) * 128],
                        state["e"][i][:, c0:512],
                        start=(i == 0), stop=(i == ni - 1),
                        skip_group_check=True)

                def tile_unit(h, i):
                    def emit():
                        if i == 0:
                            state["av"] = avp.tile([128, 512], f32, tag="av", name="av")
                            state["acc_e"] = stage.tile([128, 512], bf16,
                                                        tag="acc_e", bufs=2, name="acc_e")
                            state["e"] = {}
                        acc_av = state["av"]
                        acc_e = state["acc_e"]
                        delta = i - 4 * j
                        c0 = 128 * delta if delta > 0 else 0
                        s_ps = sp.tile([128, 512], f32, tag="s")
                        nc.tensor.matmul(
                            s_ps[:, c0:512],
                            krot[i // 4][:, (i % 4) * 128:(i % 4 + 1) * 128],
                            qrot[h][j][:, c0:512], start=True, stop=True)
                        if i > 0:
                            av_mm(h, i - 1)
                        e_sb = stage.tile([128, 512], bf16, tag="e", bufs=6)
                        # region enhance/suppress folded into exp's scale
                        if i < 5 and j >= 2:
                            nc.scalar.activation(e_sb[:, c0:512],
                                                 s_ps[:, c0:512], EXP,
                                                 scale=ksT[:, i:i + 1])
                        elif i < 5 and j == 1:
                            cs = BOUND - 512   # 99: rows >= BOUND scaled
                            nc.scalar.activation(e_sb[:, c0:cs],
                                                 s_ps[:, c0:cs], EXP)
                            nc.scalar.activation(e_sb[:, cs:512],
                                                 s_ps[:, cs:512], EXP,
                                                 scale=ksT[:, i:i + 1])
                        else:
                            nc.scalar.activation(e_sb[:, c0:512],
                                                 s_ps[:, c0:512], EXP)
                        if delta >= 0:
                            # causal mask on the partial 128-col block
                            nc.vector.tensor_mul(e_sb[:, c0:c0 + 128],
                                                 e_sb[:, c0:c0 + 128], tri[:])
                        # denominator accumulation on DVE (bf16, 2x rate)
                        if i == 0:
                            nc.vector.tensor_copy(acc_e[:], e_sb[:])
                        else:
                            nc.vector.tensor_add(acc_e[:, c0:512],
                                                 acc_e[:, c0:512],
                                                 e_sb[:, c0:512])
                        state["e"][i] = e_sb
                    return emit

                def fin_unit(h):
                    def emit():
                        acc_av = state["av"]
                        acc_e = state["acc_e"]
                        av_mm(h, ni - 1)
                        # denom replicated to all partitions via ones matmul;
                        # 1/x = exp(-ln(x)) on ACT (DVE reciprocal is 3.4us)
                        dn_ps = tmpp.tile([128, 512], f32, tag="tmp")
                        nc.tensor.matmul(dn_ps[:], ones128[:], acc_e[:],
                                         start=True, stop=True)
                        lrec = stage.tile([128, 512], f32, tag="lrec", bufs=2)
                        nc.scalar.activation(lrec[:], dn_ps[:], LN)
                        rec = stage.tile([128, 512], f32, tag="rec", bufs=2)
                        nc.scalar.activation(rec[:], lrec[:], EXP, scale=-1.0)
                        nc.vector.tensor_mul(attn[h][j][:], acc_av[:], rec[:])
                    return emit

                for h in range(QH):
                    for i in range(ni):
                        units.append(tile_unit(h, i))
                    units.append(fin_unit(h))
                return units

            def c_units(n_range, t_range, par):
                """o_proj emission units; o_ps borrows the idle accp banks."""
                units = []
                wo_t = {}

                def wo_dma(n):
                    def emit():
                        tiles = []
                        for h in range(QH):
                            t = wop.tile([128, 512], bf16, tag=f"wo{h}", bufs=2, name=f"wo{h}")
                            nc.sync.dma_start(t[:], woT[n, h])
                            tiles.append(t)
                        wo_t[n] = tiles
                    return emit

                def ct_unit(n, t_, k):
                    def emit():
                        tag = "accA" if k % 2 == 0 else "accB"
                        o_ps = accp.tile([128, 512], f32, tag=tag)
                        for h in range(QH):
                            nc.tensor.matmul(
                                o_ps[:],
                                attn[h][t_ // 4][:,
                                                 (t_ % 4) * 128:(t_ % 4 + 1) * 128],
                                wo_t[n][h][:], start=(h == 0), stop=(h == QH - 1))
                        o_sb = stage.tile([128, 512], bf16, tag="o_sb", bufs=4)
                        if k % 2 == 0:
                            nc.scalar.copy(o_sb[:], o_ps[:])
                        else:
                            nc.vector.tensor_copy(o_sb[:], o_ps[:])
                        nc.sync.dma_start(out[n, t_], o_sb[:])
                    return emit

                k = par
                n_list = list(n_range)
                units.append(wo_dma(n_list[0]))
                for ii, n in enumerate(n_list):
                    if ii + 1 < len(n_list):
                        units.append(wo_dma(n_list[ii + 1]))
                    for t_ in t_range:
                        units.append(ct_unit(n, t_, k))
                        k += 1
                return units

            # ---- emission schedule: A(0), then B(j) || A(j+1), B(3) || C ----
            for u in a_units(0, with_consts=True):
                u()
            for j in range(J):
                main = b_units(j)
                if j + 1 < J:
                    fill = a_units(j + 1)
                elif j == J - 1:
                    fill = c_units(range(8), range(12), 0)
                for u in _interleave(main, fill):
                    u()
            # o_proj tail: t 12..15 for all n (wo re-streamed)
            for u in c_units(range(7, -1, -1), range(12, 16), 0):
                u()

    # Split multi-wait instructions onto standalone EventSemaphore
    # instructions.
    import bass_rust
    bass_rust.generate_event_semaphores(nc)
    return nc


def _get_compiled():
    if "nc" not in _CACHE:
        _CACHE["nc"] = _build_bass()
        _CACHE["const"] = _host_constants()
    return _CACHE["nc"], _CACHE["const"]


def kernel(hidden_states, wq, wk, wv, wo, _trace=False):
    from concourse.bass_utils import run_bass_kernel_spmd

    nc, cst = _get_compiled()

    x = np.asarray(hidden_states, dtype=np.float32).reshape(S, D)
    xT = np.ascontiguousarray(x.T)                       # [D, S]
    xTt = _bf16(xT.reshape(DCH, 128, J, 512).transpose(2, 0, 1, 3))
    wq = np.asarray(wq, dtype=np.float32)
    wk = np.asarray(wk, dtype=np.float32)
    wv = np.asarray(wv, dtype=np.float32)
    wo = np.asarray(wo, dtype=np.float32)
    scale = 1.0 / math.sqrt(HD)

    in_maps = []
    for d in range(NCORES):
        wq_d = (wq[d * QH * HD:(d + 1) * QH * HD] * scale).T  # [D, 512]
        wk_d = wk[d * HD:(d + 1) * HD].T                      # [D, 128]
        wv_d = wv[d * HD:(d + 1) * HD].T                      # [D, 128]
        wqkv_d = np.concatenate(
            [wk_d, wv_d, wq_d], axis=1).reshape(DCH, 128, 768)
        wo_d = wo[:, d * QH * HD:(d + 1) * QH * HD].T         # [512, D]
        woT_d = np.ascontiguousarray(
            wo_d.reshape(QH, 128, 8, 512).transpose(2, 0, 1, 3))
        in_maps.append({
            "xTt": xTt,
            "wqkv": _bf16(wqkv_d),
            "woT": _bf16(woT_d),
            "cosT": cst["cosT"], "sinT": cst["sinT"],
            "sinN": cst["sinN"], "ident": cst["ident"],
            "tri": cst["tri"], "ksT": cst["ksT"],
            "ones128": cst["ones128"],
        })

    res = run_bass_kernel_spmd(nc, in_maps, core_ids=list(range(NCORES)),
                               trace=_trace)
    acc = res.results[0]["out"].astype(np.float64)
    for d in range(1, NCORES):
        acc += res.results[d]["out"].astype(np.float64)
    # out[n, t, p, f] -> out[128t+p, 512n+f]
    outp = acc.transpose(1, 2, 0, 3).reshape(S, D).astype(np.float32)
    outp = outp.reshape(1, S, D)
    if _trace:
        _CACHE["last_results"] = res
    return outp


# revision 8
# speedup vs baseline: 1.6313x; 1.0539x over previous
"""Trainium2 Bass kernel for nn_AttnAdapter: GQA attention with RoPE,
region-based enhance/suppress score scaling, causal mask, o_proj.

Sharding: tensor-parallel over heads across 8 NeuronCores. Core d holds
q-heads 4d..4d+3 (wq rows), kv-head d (wk/wv rows), and wo columns
512d..512(d+1). Each core computes a full [S, D] partial of the output;
the host sums the 8 partials (the TP all-reduce, done at unshard time).

v3: all-bf16 matmuls (FWL weight loads), weights loaded once, software-
pipelined emission: projection matmuls for seq-tile j+1 are interleaved
into the attention stream for seq-tile j (and the first o_proj tiles
into the last attention tile) so the PE never stalls on the exp stream.
Softmax denom is accumulated on DVE in bf16, replicated across
partitions with a ones-matmul, and inverted with ACT Ln/Exp (the DVE
reciprocal costs 3.4us/tile). Outputs are stored bf16 and summed on
host.
"""

import math

import numpy as np

# ---- problem constants (hardcoded; kernel.py must be self-contained) ----
S = 2048          # sequence length
D = 4096          # model dim
HD = 128          # head dim
NCORES = 8
QH = 4            # q heads per core
SYS_LEN, IMG_LEN = 35, 576
BOUND = SYS_LEN + IMG_LEN          # 611
ENH, SUP = 1.5, 0.5
ROPE_BASE = 10000.0

J = 4             # sq tiles of 512
NSK = 16          # sk tiles of 128
DCH = 32          # D chunks of 128

_CACHE = {}


def _bf16(x):
    import ml_dtypes
    return np.ascontiguousarray(np.asarray(x, dtype=ml_dtypes.bfloat16))


def _host_constants():
    inv_freq = 1.0 / (ROPE_BASE ** (np.arange(0, HD, 2, dtype=np.float32) / HD))
    pos = np.arange(S, dtype=np.float32)
    freqs = pos[:, None] * inv_freq[None, :]              # [S, 64]
    emb = np.concatenate([freqs, freqs], axis=-1)         # [S, 128]
    cosT = _bf16(np.cos(emb).T)                           # [128, S]
    sinT = _bf16(np.sin(emb).T)

    # rotate_half as a matmul: rot = R @ q (in [hd, s] layout).
    # matmul(out, lhsT, rhs) = lhsT.T @ rhs, so feed RT = R.T.
    RT = np.zeros((HD, HD), dtype=np.float32)
    half = HD // 2
    for c in range(half):
        RT[c + half, c] = -1.0      # rot[c] = -q[c+64]
    for c in range(half, HD):
        RT[c - half, c] = 1.0       # rot[c] = q[c-64]
    rmat = _bf16(RT)

    ident = np.eye(HD, dtype=np.float32)

    # Diagonal-tile causal mask [128, 128]: within the first 128-col block
    # of a diagonal tile, col c valid iff c >= p. Same for every delta.
    p = np.arange(128)[:, None]
    c = np.arange(128)[None, :]
    tri = _bf16((c >= p).astype(np.float32))

    # key_scale in partition layout per sk-tile: ksT[p, i] = scale(128*i+p)
    kpos = np.arange(S)
    key_scale = np.where(kpos < SYS_LEN, SUP,
                         np.where(kpos < BOUND, ENH, 1.0)).astype(np.float32)
    ksT = np.ascontiguousarray(key_scale[:5 * 128].reshape(5, 128).T)  # [128, 5]

    ones128 = _bf16(np.ones((HD, HD), dtype=np.float32))
    return dict(cosT=cosT, sinT=sinT, rmat=rmat, ident=ident, tri=tri,
                ksT=ksT, ones128=ones128)


def _interleave(main, fill):
    """Merge two unit lists, spreading `fill` evenly across `main`."""
    units = []
    nf = len(fill)
    nm = max(1, len(main))
    k = 0
    for m, u in enumerate(main):
        units.append(u)
        want = (m + 1) * nf // nm
        while k < want:
            units.append(fill[k])
            k += 1
    units.extend(fill[k:])
    return units


def _build_bass():
    import concourse.bass as bass
    import concourse.mybir as mybir
    from concourse.tile import TileContext
    from contextlib import ExitStack

    f32 = mybir.dt.float32
    bf16 = mybir.dt.bfloat16

    nc = bass.Bass()
    # xTt[j, d, p, f] = x.T[128d+p, 512j+f] -- each (j,d) tile contiguous
    xTt = nc.dram_tensor("xTt", [J, 16, 128, 1024], bf16, kind="ExternalInput")
    # wqkv[d, p, c]: c 0:128=wkT chunk, 128:256=wvT chunk, 256:768=wqT chunk
    wqkv = nc.dram_tensor("wqkv", [DCH, 128, 768], bf16, kind="ExternalInput")
    # woT[n, h, p, f] = woT[128h+p, 512n+f]
    woT = nc.dram_tensor("woT", [8, 128, 2048], bf16, kind="ExternalInput")
    cosT_d = nc.dram_tensor("cosT", [HD, S], bf16, kind="ExternalInput")
    sinT_d = nc.dram_tensor("sinT", [HD, S], bf16, kind="ExternalInput")
    rmat_d = nc.dram_tensor("rmat", [HD, HD], bf16, kind="ExternalInput")
    ident_d = nc.dram_tensor("ident", [HD, HD], f32, kind="ExternalInput")
    tri_d = nc.dram_tensor("tri", [HD, HD], bf16, kind="ExternalInput")
    ksT_d = nc.dram_tensor("ksT", [HD, 5], f32, kind="ExternalInput")
    ones128_d = nc.dram_tensor("ones128", [HD, HD], bf16, kind="ExternalInput")
    # out[n, t, p, f] = out[128t+p, 512n+f], bf16 partial (host sums cores)
    out = nc.dram_tensor("out", [8, 4, 128, 2048], bf16, kind="ExternalOutput")

    EXP = mybir.ActivationFunctionType.Exp
    LN = mybir.ActivationFunctionType.Ln

    with TileContext(nc) as tc, ExitStack() as ctx:
        const = ctx.enter_context(tc.tile_pool(name="const", bufs=1))
        wpool = ctx.enter_context(tc.tile_pool(name="wpool", bufs=1))
        persist = ctx.enter_context(tc.tile_pool(name="persist", bufs=1))

        wt = [wpool.tile([128, 768], bf16, name=f"wt{d}") for d in range(DCH)]
        qrot = [[persist.tile([HD, 512], bf16, name=f"qrot{m}_{j}")
                 for j in range(J)] for m in range(QH)]
        krot = [persist.tile([HD, 512], bf16, name=f"krot{j}") for j in range(J)]
        vnat = [persist.tile([HD, 512], bf16, name=f"vnat{j}") for j in range(J)]
        attn = [[persist.tile([HD, 512], bf16, name=f"attn{h}_{j}")
                 for j in range(J)] for h in range(QH)]

        cosT = const.tile([HD, S], bf16)
        sinT = const.tile([HD, S], bf16)
        rmat = const.tile([HD, HD], bf16)
        ident = const.tile([HD, HD], f32)
        tri = const.tile([HD, HD], bf16)
        ksT = const.tile([HD, 5], f32)
        ones128 = const.tile([HD, HD], bf16)

        with tc.tile_pool(name="xp", bufs=2) as xp, \
             tc.tile_pool(name="wop", bufs=1) as wop, \
             tc.tile_pool(name="accp", bufs=1, space="PSUM") as accp, \
             tc.tile_pool(name="tmpp", bufs=1, space="PSUM") as tmpp, \
             tc.tile_pool(name="sp", bufs=3, space="PSUM") as sp, \
             tc.tile_pool(name="avp", bufs=2, space="PSUM") as avp, \
             tc.tile_pool(name="stage", bufs=1) as stage:

            xt = [[None] * DCH for _ in range(J)]

            def rope_unit(acc, dst, j, eng):
                def emit():
                    sq = slice(j * 512, (j + 1) * 512)
                    q_sb = stage.tile([128, 512], bf16, tag="q_sb", bufs=3)
                    # release copy frees the acc bank; alternate queues so it
                    # is not stuck behind the exp stream
                    if eng == 0:
                        nc.scalar.copy(q_sb[:], acc[:])
                    else:
                        nc.vector.tensor_copy(q_sb[:], acc[:])
                    rot_ps = tmpp.tile([128, 512], f32, tag="tmp")
                    nc.tensor.matmul(rot_ps[:], rmat[:], q_sb[:],
                                     start=True, stop=True)
                    t2 = stage.tile([128, 512], f32, tag="t2", bufs=2)
                    nc.vector.tensor_mul(t2[:], rot_ps[:], sinT[:, sq])
                    nc.vector.tensor_mul(dst[:], q_sb[:], cosT[:, sq])
                    nc.vector.tensor_add(dst[:], dst[:], t2[:])
                return emit

            def a_units(j, with_consts=False):
                """Projection+RoPE units for seq-tile j: (dma_units, main)."""
                dma_units = []
                units = []
                accs = {}

                def dma_unit(dp):
                    def emit():
                        if j == 0:
                            nc.sync.dma_start(wt[2 * dp][:, 0:256],
                                              wqkv[2 * dp, :, 0:256])
                            nc.sync.dma_start(wt[2 * dp + 1][:, 0:256],
                                              wqkv[2 * dp + 1, :, 0:256])
                        t = xp.tile([128, 1024], bf16, tag=f"x{dp}",
                                    bufs=2, name=f"x{dp}")
                        nc.sync.dma_start(t[:], xTt[j, dp])
                        xt[j][2 * dp] = t[:, 0:512]
                        xt[j][2 * dp + 1] = t[:, 512:1024]
                    return emit

                def wq_dma_unit(d):
                    def emit():
                        nc.sync.dma_start(wt[d][:, 256:768],
                                          wqkv[d, :, 256:768])
                    return emit

                def mm_unit(g, d):
                    def emit():
                        if d == 0:
                            accs[g] = (accp.tile([128, 512], f32, tag="accA", name="accA"),
                                       accp.tile([128, 512], f32, tag="accB", name="accB"))
                        accA, accB = accs[g]
                        ca = g * 256
                        st = (d == 0)
                        sp_ = (d == DCH - 1)
                        nc.tensor.matmul(accA[:], wt[d][:, ca:ca + 128],
                                         xt[j][d][:], start=st, stop=sp_)
                        nc.tensor.matmul(accB[:], wt[d][:, ca + 128:ca + 256],
                                         xt[j][d][:], start=st, stop=sp_)
                    return emit

                def vt_unit(g):
                    def emit():
                        accB = accs[g][1]
                        v_sb = stage.tile([128, 512], f32, tag="v_sb", bufs=2)
                        nc.scalar.copy(v_sb[:], accB[:])
                        vt_ps = tmpp.tile([128, 512], f32, tag="tmp")
                        for b in range(4):
                            nc.tensor.transpose(
                                vt_ps[:, b * 128:(b + 1) * 128],
                                v_sb[:, b * 128:(b + 1) * 128], ident[:])
                        nc.vector.tensor_copy(vnat[j][:], vt_ps[:])
                    return emit

                def rope_lazy(g, which, dst, eng):
                    def emit():
                        rope_unit(accs[g][which], dst, j, eng)()
                    return emit

                for dp in range(16):
                    dma_units.append(dma_unit(dp))
                if with_consts:
                    def cdma():
                        nc.sync.dma_start(cosT[:], cosT_d[:, :])
                        nc.sync.dma_start(sinT[:], sinT_d[:, :])
                        nc.sync.dma_start(rmat[:], rmat_d[:, :])
                        nc.sync.dma_start(ident[:], ident_d[:, :])
                        nc.sync.dma_start(tri[:], tri_d[:, :])
                        nc.sync.dma_start(ksT[:], ksT_d[:, :])
                        nc.sync.dma_start(ones128[:], ones128_d[:, :])
                    dma_units.append(cdma)
                    for d in range(DCH):
                        dma_units.append(wq_dma_unit(d))
                # group 0: (k, v); group 1: (q0, q1); group 2: (q2, q3)
                for d in range(DCH):
                    units.append(mm_unit(0, d))
                units.append(rope_lazy(0, 0, krot[j], 0))
                units.append(vt_unit(0))
                for d in range(DCH):
                    units.append(mm_unit(1, d))
                units.append(rope_lazy(1, 0, qrot[0][j], 1))
                units.append(rope_lazy(1, 1, qrot[1][j], 0))
                for d in range(DCH):
                    units.append(mm_unit(2, d))
                units.append(rope_lazy(2, 0, qrot[2][j], 1))
                units.append(rope_lazy(2, 1, qrot[3][j], 0))
                return dma_units, units

            def b_units(j):
                """Attention emission units for seq-tile j."""
                units = []
                ni = 4 * j + 4
                state = {}

                def av_mm(h, i):
                    delta = i - 4 * j
                    c0 = 128 * delta if delta > 0 else 0
                    nc.tensor.matmul(
                        state["av"][:, c0:512],
                        vnat[i // 4][:, (i % 4) * 128:(i % 4 + 1) * 128],
                        state["e"][i][:, c0:512],
                        start=(i == 0), stop=(i == ni - 1),
                        skip_group_check=True)

                def tile_unit(h, i):
                    def emit():
                        if i == 0:
                            state["av"] = avp.tile([128, 512], f32, tag="av", name="av")
                            state["acc_e"] = stage.tile([128, 512], bf16,
                                                        tag="acc_e", bufs=2, name="acc_e")
                            state["e"] = {}
                        acc_av = state["av"]
                        acc_e = state["acc_e"]
                        delta = i - 4 * j
                        c0 = 128 * delta if delta > 0 else 0
                        s_ps = sp.tile([128, 512], f32, tag="s")
                        nc.tensor.matmul(
                            s_ps[:, c0:512],
                            krot[i // 4][:, (i % 4) * 128:(i % 4 + 1) * 128],
                            qrot[h][j][:, c0:512], start=True, stop=True)
                        if i > 0:
                            av_mm(h, i - 1)
                        e_sb = stage.tile([128, 512], bf16, tag="e", bufs=6)
                        # region enhance/suppress folded into exp's scale
                        if i < 5 and j >= 2:
                            nc.scalar.activation(e_sb[:, c0:512],
                                                 s_ps[:, c0:512], EXP,
                                                 scale=ksT[:, i:i + 1])
                        elif i < 5 and j == 1:
                            cs = BOUND - 512   # 99: rows >= BOUND scaled
                            nc.scalar.activation(e_sb[:, c0:cs],
                                                 s_ps[:, c0:cs], EXP)
                            nc.scalar.activation(e_sb[:, cs:512],
                                                 s_ps[:, cs:512], EXP,
                                                 scale=ksT[:, i:i + 1])
                        else:
                            nc.scalar.activation(e_sb[:, c0:512],
                                                 s_ps[:, c0:512], EXP)
                        if delta >= 0:
                            # causal mask on the partial 128-col block
                            nc.vector.tensor_mul(e_sb[:, c0:c0 + 128],
                                                 e_sb[:, c0:c0 + 128], tri[:])
                        # denominator accumulation on DVE (bf16, 2x rate)
                        if i == 0:
                            nc.vector.tensor_copy(acc_e[:], e_sb[:])
                        else:
                            nc.vector.tensor_add(acc_e[:, c0:512],
                                                 acc_e[:, c0:512],
                                                 e_sb[:, c0:512])
                        state["e"][i] = e_sb
                    return emit

                def fin_unit(h):
                    def emit():
                        acc_av = state["av"]
                        acc_e = state["acc_e"]
                        av_mm(h, ni - 1)
                        # denom replicated to all partitions via ones matmul;
                        # 1/x = exp(-ln(x)) on ACT (DVE reciprocal is 3.4us)
                        dn_ps = tmpp.tile([128, 512], f32, tag="tmp")
                        nc.tensor.matmul(dn_ps[:], ones128[:], acc_e[:],
                                         start=True, stop=True)
                        lrec = stage.tile([128, 512], f32, tag="lrec", bufs=2)
                        nc.scalar.activation(lrec[:], dn_ps[:], LN)
                        rec = stage.tile([128, 512], f32, tag="rec", bufs=2)
                        nc.scalar.activation(rec[:], lrec[:], EXP, scale=-1.0)
                        nc.vector.tensor_mul(attn[h][j][:], acc_av[:], rec[:])
                    return emit

                for h in range(QH):
                    for i in range(ni):
                        units.append(tile_unit(h, i))
                    units.append(fin_unit(h))
                return units

            def c_units(n_range, t_range, par):
                """o_proj emission units; o_ps borrows the idle accp banks."""
                units = []
                wo_t = {}

                def wo_dma(n):
                    def emit():
                        t = wop.tile([128, 2048], bf16, tag="wo", bufs=2,
                                     name="wo")
                        nc.sync.dma_start(t[:], woT[n])
                        wo_t[n] = t
                    return emit

                o_quad = {}

                def ct_unit(n, t_, k):
                    def emit():
                        ti = t_ % 4
                        if ti == 0:
                            o_quad[0] = stage.tile([128, 2048], bf16,
                                                   tag="o_sb", bufs=2,
                                                   name="o_sb")
                        o_sb = o_quad[0]
                        tag = "accA" if k % 2 == 0 else "accB"
                        o_ps = accp.tile([128, 512], f32, tag=tag)
                        for h in range(QH):
                            nc.tensor.matmul(
                                o_ps[:],
                                attn[h][t_ // 4][:,
                                                 (t_ % 4) * 128:(t_ % 4 + 1) * 128],
                                wo_t[n][:, h * 512:(h + 1) * 512],
                                start=(h == 0), stop=(h == QH - 1))
                        dst = o_sb[:, ti * 512:(ti + 1) * 512]
                        if k % 2 == 0:
                            nc.scalar.copy(dst, o_ps[:])
                        else:
                            nc.vector.tensor_copy(dst, o_ps[:])
                        if ti == 3:
                            nc.sync.dma_start(out[n, t_ // 4], o_sb[:])
                    return emit

                k = par
                n_list = list(n_range)
                units.append(wo_dma(n_list[0]))
                for ii, n in enumerate(n_list):
                    if ii + 1 < len(n_list):
                        units.append(wo_dma(n_list[ii + 1]))
                    for t_ in t_range:
                        units.append(ct_unit(n, t_, k))
                        k += 1
                return units

            # ---- emission schedule: A(0), then B(j) || A(j+1), B(3) || C ----
            d0, m0 = a_units(0, with_consts=True)
            for u in d0 + m0:
                u()
            for j in range(J):
                main = b_units(j)
                if j + 1 < J:
                    dma_f, fill = a_units(j + 1)
                else:
                    dma_f, fill = [], c_units(range(8), range(12), 0)
                hold = min(6, len(main) - 1)
                for u in dma_f + main[:hold] + _interleave(main[hold:], fill):
                    u()
            # o_proj tail: t 12..15 for all n (wo re-streamed)
            for u in c_units(range(7, -1, -1), range(12, 16), 0):
                u()

    # Split multi-wait instructions onto standalone EventSemaphore
    # instructions.
    import bass_rust
    bass_rust.generate_event_semaphores(nc)
    return nc


def _get_compiled():
    if "nc" not in _CACHE:
        _CACHE["nc"] = _build_bass()
        _CACHE["const"] = _host_constants()
    return _CACHE["nc"], _CACHE["const"]


def kernel(hidden_states, wq, wk, wv, wo, _trace=False):
    from concourse.bass_utils import run_bass_kernel_spmd

    nc, cst = _get_compiled()

    x = np.asarray(hidden_states, dtype=np.float32).reshape(S, D)
    xT = np.ascontiguousarray(x.T)                       # [D, S]
    # [j, dpair, p, (dlo f | dhi f)] -- 2KB partition lines per DMA
    xTt = xT.reshape(16, 2, 128, J, 512).transpose(3, 0, 2, 1, 4)
    xTt = _bf16(xTt.reshape(J, 16, 128, 1024))
    wq = np.asarray(wq, dtype=np.float32)
    wk = np.asarray(wk, dtype=np.float32)
    wv = np.asarray(wv, dtype=np.float32)
    wo = np.asarray(wo, dtype=np.float32)
    scale = 1.0 / math.sqrt(HD)

    in_maps = []
    for d in range(NCORES):
        wq_d = (wq[d * QH * HD:(d + 1) * QH * HD] * scale).T  # [D, 512]
        wk_d = wk[d * HD:(d + 1) * HD].T                      # [D, 128]
        wv_d = wv[d * HD:(d + 1) * HD].T                      # [D, 128]
        wqkv_d = np.concatenate(
            [wk_d, wv_d, wq_d], axis=1).reshape(DCH, 128, 768)
        wo_d = wo[:, d * QH * HD:(d + 1) * QH * HD].T         # [512, D]
        # [n, p, (h0 f | h1 f | h2 f | h3 f)] -- one DMA per n
        woT_d = np.ascontiguousarray(
            wo_d.reshape(QH, 128, 8, 512).transpose(2, 1, 0, 3).reshape(
                8, 128, 2048))
        in_maps.append({
            "xTt": xTt,
            "wqkv": _bf16(wqkv_d),
            "woT": _bf16(woT_d),
            "cosT": cst["cosT"], "sinT": cst["sinT"],
            "rmat": cst["rmat"], "ident": cst["ident"],
            "tri": cst["tri"], "ksT": cst["ksT"],
            "ones128": cst["ones128"],
        })

    res = run_bass_kernel_spmd(nc, in_maps, core_ids=list(range(NCORES)),
                               trace=_trace)
    acc = res.results[0]["out"].astype(np.float64)
    for d in range(1, NCORES):
        acc += res.results[d]["out"].astype(np.float64)
    # out[n, tq, p, ti*512+f] -> out[128*(4tq+ti)+p, 512n+f]
    acc = acc.reshape(8, 4, 128, 4, 512)        # [n, tq, p, ti, f]
    outp = acc.transpose(1, 3, 2, 0, 4).reshape(S, D).astype(np.float32)
    outp = outp.reshape(1, S, D)
    if _trace:
        _CACHE["last_results"] = res
    return outp


# revision 9
# speedup vs baseline: 1.7428x; 1.0684x over previous
"""Trainium2 Bass kernel for nn_AttnAdapter: GQA attention with RoPE,
region-based enhance/suppress score scaling, causal mask, o_proj.

Sharding: tensor-parallel over heads across 8 NeuronCores. Core d holds
q-heads 4d..4d+3 (wq rows), kv-head d (wk/wv rows), and wo columns
512d..512(d+1). Each core computes a full [S, D] partial of the output;
the host sums the 8 partials (the TP all-reduce, done at unshard time).

v3: all-bf16 matmuls (FWL weight loads), weights loaded once, software-
pipelined emission: projection matmuls for seq-tile j+1 are interleaved
into the attention stream for seq-tile j (and the first o_proj tiles
into the last attention tile) so the PE never stalls on the exp stream.
Softmax denom is accumulated on DVE in bf16, replicated across
partitions with a ones-matmul, and inverted with ACT Ln/Exp (the DVE
reciprocal costs 3.4us/tile). Outputs are stored bf16 and summed on
host.
"""

import math

import numpy as np

# ---- problem constants (hardcoded; kernel.py must be self-contained) ----
S = 2048          # sequence length
D = 4096          # model dim
HD = 128          # head dim
NCORES = 8
QH = 4            # q heads per core
SYS_LEN, IMG_LEN = 35, 576
BOUND = SYS_LEN + IMG_LEN          # 611
ENH, SUP = 1.5, 0.5
ROPE_BASE = 10000.0

J = 4             # sq tiles of 512
NSK = 16          # sk tiles of 128
DCH = 32          # D chunks of 128

_CACHE = {}


def _bf16(x):
    import ml_dtypes
    return np.ascontiguousarray(np.asarray(x, dtype=ml_dtypes.bfloat16))


def _host_constants():
    inv_freq = 1.0 / (ROPE_BASE ** (np.arange(0, HD, 2, dtype=np.float32) / HD))
    pos = np.arange(S, dtype=np.float32)
    freqs = pos[:, None] * inv_freq[None, :]              # [S, 64]
    emb = np.concatenate([freqs, freqs], axis=-1)         # [S, 128]
    cosT = _bf16(np.cos(emb).T)                           # [128, S]
    sinT = _bf16(np.sin(emb).T)

    # rotate_half as a matmul: rot = R @ q (in [hd, s] layout).
    # matmul(out, lhsT, rhs) = lhsT.T @ rhs, so feed RT = R.T.
    RT = np.zeros((HD, HD), dtype=np.float32)
    half = HD // 2
    for c in range(half):
        RT[c + half, c] = -1.0      # rot[c] = -q[c+64]
    for c in range(half, HD):
        RT[c - half, c] = 1.0       # rot[c] = q[c-64]
    rmat = _bf16(RT)

    ident = np.eye(HD, dtype=np.float32)

    # Diagonal-tile causal mask [128, 128]: within the first 128-col block
    # of a diagonal tile, col c valid iff c >= p. Same for every delta.
    p = np.arange(128)[:, None]
    c = np.arange(128)[None, :]
    tri = _bf16((c >= p).astype(np.float32))

    # key_scale in partition layout per sk-tile: ksT[p, i] = scale(128*i+p)
    kpos = np.arange(S)
    key_scale = np.where(kpos < SYS_LEN, SUP,
                         np.where(kpos < BOUND, ENH, 1.0)).astype(np.float32)
    ksT = np.ascontiguousarray(key_scale[:5 * 128].reshape(5, 128).T)  # [128, 5]

    ones128 = _bf16(np.ones((HD, HD), dtype=np.float32))
    return dict(cosT=cosT, sinT=sinT, rmat=rmat, ident=ident, tri=tri,
                ksT=ksT, ones128=ones128)


def _interleave(main, fill):
    """Merge two unit lists, spreading `fill` evenly across `main`."""
    units = []
    nf = len(fill)
    nm = max(1, len(main))
    k = 0
    for m, u in enumerate(main):
        units.append(u)
        want = (m + 1) * nf // nm
        while k < want:
            units.append(fill[k])
            k += 1
    units.extend(fill[k:])
    return units


def _build_bass():
    import concourse.bass as bass
    import concourse.mybir as mybir
    from concourse.tile import TileContext
    from contextlib import ExitStack

    f32 = mybir.dt.float32
    bf16 = mybir.dt.bfloat16

    nc = bass.Bass()
    # xTt[j, d, p, f] = x.T[128d+p, 512j+f] -- each (j,d) tile contiguous
    xTt = nc.dram_tensor("xTt", [J, 8, 128, 2048], bf16, kind="ExternalInput")
    # wkv4[q, p, l*256+c]: c 0:128=wkT chunk of d=4q+l, 128:256=wvT chunk
    wkv4 = nc.dram_tensor("wkv4", [8, 128, 1024], bf16, kind="ExternalInput")
    # wq4[q, p, l*512+c]: wqT chunk (4 heads x 128) of d=4q+l
    wq4 = nc.dram_tensor("wq4", [8, 128, 2048], bf16, kind="ExternalInput")
    # woT[n, h, p, f] = woT[128h+p, 512n+f]
    woT = nc.dram_tensor("woT", [8, 128, 2048], bf16, kind="ExternalInput")
    cosT_d = nc.dram_tensor("cosT", [HD, S], bf16, kind="ExternalInput")
    sinT_d = nc.dram_tensor("sinT", [HD, S], bf16, kind="ExternalInput")
    rmat_d = nc.dram_tensor("rmat", [HD, HD], bf16, kind="ExternalInput")
    ident_d = nc.dram_tensor("ident", [HD, HD], f32, kind="ExternalInput")
    tri_d = nc.dram_tensor("tri", [HD, HD], bf16, kind="ExternalInput")
    ksT_d = nc.dram_tensor("ksT", [HD, 5], f32, kind="ExternalInput")
    ones128_d = nc.dram_tensor("ones128", [HD, HD], bf16, kind="ExternalInput")
    # out[n, t, p, f] = out[128t+p, 512n+f], bf16 partial (host sums cores)
    out = nc.dram_tensor("out", [8, 4, 128, 2048], bf16, kind="ExternalOutput")

    EXP = mybir.ActivationFunctionType.Exp
    LN = mybir.ActivationFunctionType.Ln

    with TileContext(nc) as tc, ExitStack() as ctx:
        const = ctx.enter_context(tc.tile_pool(name="const", bufs=1))
        wpool = ctx.enter_context(tc.tile_pool(name="wpool", bufs=1))
        persist = ctx.enter_context(tc.tile_pool(name="persist", bufs=1))

        wkv = [wpool.tile([128, 1024], bf16, name=f"wkv{q}") for q in range(8)]
        wqt = [wpool.tile([128, 2048], bf16, name=f"wqt{q}") for q in range(8)]
        qrot = [[persist.tile([HD, 512], bf16, name=f"qrot{m}_{j}")
                 for j in range(J)] for m in range(QH)]
        krot = [persist.tile([HD, 512], bf16, name=f"krot{j}") for j in range(J)]
        vnat = [persist.tile([HD, 512], bf16, name=f"vnat{j}") for j in range(J)]
        attn = [[persist.tile([HD, 512], bf16, name=f"attn{h}_{j}")
                 for j in range(J)] for h in range(QH)]

        cosT = const.tile([HD, S], bf16)
        sinT = const.tile([HD, S], bf16)
        rmat = const.tile([HD, HD], bf16)
        ident = const.tile([HD, HD], f32)
        tri = const.tile([HD, HD], bf16)
        ksT = const.tile([HD, 5], f32)
        ones128 = const.tile([HD, HD], bf16)

        with tc.tile_pool(name="xp", bufs=2) as xp, \
             tc.tile_pool(name="wop", bufs=1) as wop, \
             tc.tile_pool(name="accp", bufs=1, space="PSUM") as accp, \
             tc.tile_pool(name="tmpp", bufs=1, space="PSUM") as tmpp, \
             tc.tile_pool(name="sp", bufs=3, space="PSUM") as sp, \
             tc.tile_pool(name="avp", bufs=2, space="PSUM") as avp, \
             tc.tile_pool(name="stage", bufs=1) as stage:

            xt = [[None] * DCH for _ in range(J)]

            def rope_unit(acc, dst, j, eng):
                def emit():
                    sq = slice(j * 512, (j + 1) * 512)
                    q_sb = stage.tile([128, 512], bf16, tag="q_sb", bufs=3)
                    # release copy frees the acc bank; alternate queues so it
                    # is not stuck behind the exp stream
                    if eng == 0:
                        nc.scalar.copy(q_sb[:], acc[:])
                    else:
                        nc.vector.tensor_copy(q_sb[:], acc[:])
                    rot_ps = tmpp.tile([128, 512], f32, tag="tmp")
                    nc.tensor.matmul(rot_ps[:], rmat[:], q_sb[:],
                                     start=True, stop=True)
                    t2 = stage.tile([128, 512], f32, tag="t2", bufs=2)
                    nc.vector.tensor_mul(t2[:], rot_ps[:], sinT[:, sq])
                    nc.vector.tensor_mul(dst[:], q_sb[:], cosT[:, sq])
                    nc.vector.tensor_add(dst[:], dst[:], t2[:])
                return emit

            def a_units(j, with_consts=False):
                """Projection+RoPE units for seq-tile j: (dma_units, main)."""
                dma_units = []
                units = []
                accs = {}

                def dma_unit(q):
                    def emit():
                        if j == 0:
                            nc.sync.dma_start(wkv[q][:], wkv4[q])
                        t = xp.tile([128, 2048], bf16, tag=f"x{q}",
                                    bufs=2, name=f"x{q}")
                        nc.sync.dma_start(t[:], xTt[j, q])
                        for l in range(4):
                            xt[j][4 * q + l] = t[:, l * 512:(l + 1) * 512]
                    return emit

                def wq_dma_unit(q):
                    def emit():
                        nc.sync.dma_start(wqt[q][:], wq4[q])
                    return emit

                def mm_unit(g, d):
                    def emit():
                        if d == 0:
                            accs[g] = (accp.tile([128, 512], f32, tag="accA", name="accA"),
                                       accp.tile([128, 512], f32, tag="accB", name="accB"))
                        accA, accB = accs[g]
                        q, l = d // 4, d % 4
                        if g == 0:
                            wA = wkv[q][:, l * 256:l * 256 + 128]
                            wB = wkv[q][:, l * 256 + 128:l * 256 + 256]
                        else:
                            m = 2 * (g - 1)
                            wA = wqt[q][:, l * 512 + m * 128:l * 512 + (m + 1) * 128]
                            wB = wqt[q][:, l * 512 + (m + 1) * 128:l * 512 + (m + 2) * 128]
                        st = (d == 0)
                        sp_ = (d == DCH - 1)
                        nc.tensor.matmul(accA[:], wA, xt[j][d][:],
                                         start=st, stop=sp_)
                        nc.tensor.matmul(accB[:], wB, xt[j][d][:],
                                         start=st, stop=sp_)
                    return emit

                def vt_unit(g):
                    def emit():
                        accB = accs[g][1]
                        v_sb = stage.tile([128, 512], f32, tag="v_sb", bufs=2)
                        nc.scalar.copy(v_sb[:], accB[:])
                        vt_ps = tmpp.tile([128, 512], f32, tag="tmp")
                        for b in range(4):
                            nc.tensor.transpose(
                                vt_ps[:, b * 128:(b + 1) * 128],
                                v_sb[:, b * 128:(b + 1) * 128], ident[:])
                        nc.vector.tensor_copy(vnat[j][:], vt_ps[:])
                    return emit

                def rope_lazy(g, which, dst, eng):
                    def emit():
                        rope_unit(accs[g][which], dst, j, eng)()
                    return emit

                for q in range(8):
                    dma_units.append(dma_unit(q))
                if with_consts:
                    def cdma():
                        nc.sync.dma_start(cosT[:], cosT_d[:, :])
                        nc.sync.dma_start(sinT[:], sinT_d[:, :])
                        nc.sync.dma_start(rmat[:], rmat_d[:, :])
                        nc.sync.dma_start(ident[:], ident_d[:, :])
                        nc.sync.dma_start(tri[:], tri_d[:, :])
                        nc.sync.dma_start(ksT[:], ksT_d[:, :])
                        nc.sync.dma_start(ones128[:], ones128_d[:, :])
                    dma_units.append(cdma)
                    for q in range(8):
                        dma_units.append(wq_dma_unit(q))
                # group 0: (k, v); group 1: (q0, q1); group 2: (q2, q3)
                for d in range(DCH):
                    units.append(mm_unit(0, d))
                units.append(rope_lazy(0, 0, krot[j], 0))
                units.append(vt_unit(0))
                for d in range(DCH):
                    units.append(mm_unit(1, d))
                units.append(rope_lazy(1, 0, qrot[0][j], 1))
                units.append(rope_lazy(1, 1, qrot[1][j], 0))
                for d in range(DCH):
                    units.append(mm_unit(2, d))
                units.append(rope_lazy(2, 0, qrot[2][j], 1))
                units.append(rope_lazy(2, 1, qrot[3][j], 0))
                return dma_units, units

            def b_units(j):
                """Attention emission units for seq-tile j."""
                units = []
                ni = 4 * j + 4
                state = {}

                def av_mm(h, i):
                    delta = i - 4 * j
                    c0 = 128 * delta if delta > 0 else 0
                    nc.tensor.matmul(
                        state["av"][:, c0:512],
                        vnat[i // 4][:, (i % 4) * 128:(i % 4 + 1) * 128],
                        state["e"][i][:, c0:512],
                        start=(i == 0), stop=(i == ni - 1),
                        skip_group_check=True)

                def tile_unit(h, i):
                    def emit():
                        if i == 0:
                            state["av"] = avp.tile([128, 512], f32, tag="av", name="av")
                            state["acc_e"] = stage.tile([128, 512], bf16,
                                                        tag="acc_e", bufs=2, name="acc_e")
                            state["e"] = {}
                        acc_av = state["av"]
                        acc_e = state["acc_e"]
                        delta = i - 4 * j
                        c0 = 128 * delta if delta > 0 else 0
                        s_ps = sp.tile([128, 512], f32, tag="s")
                        nc.tensor.matmul(
                            s_ps[:, c0:512],
                            krot[i // 4][:, (i % 4) * 128:(i % 4 + 1) * 128],
                            qrot[h][j][:, c0:512], start=True, stop=True)
                        if i > 0:
                            av_mm(h, i - 1)
                        e_sb = stage.tile([128, 512], bf16, tag="e", bufs=6)
                        # region enhance/suppress folded into exp's scale
                        if i < 5 and j >= 2:
                            nc.scalar.activation(e_sb[:, c0:512],
                                                 s_ps[:, c0:512], EXP,
                                                 scale=ksT[:, i:i + 1])
                        elif i < 5 and j == 1:
                            cs = BOUND - 512   # 99: rows >= BOUND scaled
                            nc.scalar.activation(e_sb[:, c0:cs],
                                                 s_ps[:, c0:cs], EXP)
                            nc.scalar.activation(e_sb[:, cs:512],
                                                 s_ps[:, cs:512], EXP,
                                                 scale=ksT[:, i:i + 1])
                        else:
                            nc.scalar.activation(e_sb[:, c0:512],
                                                 s_ps[:, c0:512], EXP)
                        if delta >= 0:
                            # causal mask on the partial 128-col block
                            nc.vector.tensor_mul(e_sb[:, c0:c0 + 128],
                                                 e_sb[:, c0:c0 + 128], tri[:])
                        # denominator accumulation on DVE (bf16, 2x rate)
                        if i == 0:
                            nc.vector.tensor_copy(acc_e[:], e_sb[:])
                        else:
                            nc.vector.tensor_add(acc_e[:, c0:512],
                                                 acc_e[:, c0:512],
                                                 e_sb[:, c0:512])
                        state["e"][i] = e_sb
                    return emit

                def fin_unit(h):
                    def emit():
                        acc_av = state["av"]
                        acc_e = state["acc_e"]
                        av_mm(h, ni - 1)
                        # denom replicated to all partitions via ones matmul;
                        # 1/x = exp(-ln(x)) on ACT (DVE reciprocal is 3.4us)
                        dn_ps = tmpp.tile([128, 512], f32, tag="tmp")
                        nc.tensor.matmul(dn_ps[:], ones128[:], acc_e[:],
                                         start=True, stop=True)
                        lrec = stage.tile([128, 512], f32, tag="lrec", bufs=2)
                        nc.scalar.activation(lrec[:], dn_ps[:], LN)
                        rec = stage.tile([128, 512], f32, tag="rec", bufs=2)
                        nc.scalar.activation(rec[:], lrec[:], EXP, scale=-1.0)
                        nc.vector.tensor_mul(attn[h][j][:], acc_av[:], rec[:])
                    return emit

                for h in range(QH):
                    for i in range(ni):
                        units.append(tile_unit(h, i))
                    units.append(fin_unit(h))
                return units

            def c_units(n_range, t_range, par):
                """o_proj emission units; o_ps borrows the idle accp banks."""
                units = []
                wo_t = {}

                def wo_dma(n):
                    def emit():
                        t = wop.tile([128, 2048], bf16, tag="wo", bufs=2,
                                     name="wo")
                        nc.sync.dma_start(t[:], woT[n])
                        wo_t[n] = t
                    return emit

                o_quad = {}

                def ct_unit(n, t_, k):
                    def emit():
                        ti = t_ % 4
                        if ti == 0:
                            o_quad[0] = stage.tile([128, 2048], bf16,
                                                   tag="o_sb", bufs=2,
                                                   name="o_sb")
                        o_sb = o_quad[0]
                        tag = "accA" if k % 2 == 0 else "accB"
                        o_ps = accp.tile([128, 512], f32, tag=tag)
                        for h in range(QH):
                            nc.tensor.matmul(
                                o_ps[:],
                                attn[h][t_ // 4][:,
                                                 (t_ % 4) * 128:(t_ % 4 + 1) * 128],
                                wo_t[n][:, h * 512:(h + 1) * 512],
                                start=(h == 0), stop=(h == QH - 1))
                        dst = o_sb[:, ti * 512:(ti + 1) * 512]
                        if k % 2 == 0:
                            nc.scalar.copy(dst, o_ps[:])
                        else:
                            nc.vector.tensor_copy(dst, o_ps[:])
                        if ti == 3:
                            nc.sync.dma_start(out[n, t_ // 4], o_sb[:])
                    return emit

                k = par
                n_list = list(n_range)
                units.append(wo_dma(n_list[0]))
                for ii, n in enumerate(n_list):
                    if ii + 1 < len(n_list):
                        units.append(wo_dma(n_list[ii + 1]))
                    for t_ in t_range:
                        units.append(ct_unit(n, t_, k))
                        k += 1
                return units

            # ---- emission schedule: A(0), then B(j) || A(j+1), B(3) || C ----
            d0, m0 = a_units(0, with_consts=True)
            for u in d0 + m0:
                u()
            for j in range(J):
                main = b_units(j)
                if j + 1 < J:
                    dma_f, fill = a_units(j + 1)
                else:
                    dma_f, fill = [], c_units(range(8), range(12), 0)
                hold = min(6, len(main) - 1)
                for u in dma_f + main[:hold] + _interleave(main[hold:], fill):
                    u()
            # o_proj tail: t 12..15 for all n (wo re-streamed)
            for u in c_units(range(7, -1, -1), range(12, 16), 0):
                u()

    # Split multi-wait instructions onto standalone EventSemaphore
    # instructions.
    import bass_rust
    bass_rust.generate_event_semaphores(nc)
    return nc


def _get_compiled():
    if "nc" not in _CACHE:
        _CACHE["nc"] = _build_bass()
        _CACHE["const"] = _host_constants()
    return _CACHE["nc"], _CACHE["const"]


def kernel(hidden_states, wq, wk, wv, wo, _trace=False):
    from concourse.bass_utils import run_bass_kernel_spmd

    nc, cst = _get_compiled()

    x = np.asarray(hidden_states, dtype=np.float32).reshape(S, D)
    xT = np.ascontiguousarray(x.T)                       # [D, S]
    # [j, dquad, p, (d0 f | d1 f | d2 f | d3 f)] -- 4KB lines, one DMA each
    xTt = xT.reshape(8, 4, 128, J, 512).transpose(3, 0, 2, 1, 4)
    xTt = _bf16(xTt.reshape(J, 8, 128, 2048))
    wq = np.asarray(wq, dtype=np.float32)
    wk = np.asarray(wk, dtype=np.float32)
    wv = np.asarray(wv, dtype=np.float32)
    wo = np.asarray(wo, dtype=np.float32)
    scale = 1.0 / math.sqrt(HD)

    in_maps = []
    for d in range(NCORES):
        wq_d = (wq[d * QH * HD:(d + 1) * QH * HD] * scale).T  # [D, 512]
        wk_d = wk[d * HD:(d + 1) * HD].T                      # [D, 128]
        wv_d = wv[d * HD:(d + 1) * HD].T                      # [D, 128]
        wkv_d = np.concatenate([wk_d, wv_d], axis=1)       # [D, 256]
        wkv4_d = wkv_d.reshape(8, 4, 128, 256).transpose(0, 2, 1, 3).reshape(
            8, 128, 1024)
        wq4_d = wq_d.reshape(8, 4, 128, 512).transpose(0, 2, 1, 3).reshape(
            8, 128, 2048)
        wo_d = wo[:, d * QH * HD:(d + 1) * QH * HD].T         # [512, D]
        # [n, p, (h0 f | h1 f | h2 f | h3 f)] -- one DMA per n
        woT_d = np.ascontiguousarray(
            wo_d.reshape(QH, 128, 8, 512).transpose(2, 1, 0, 3).reshape(
                8, 128, 2048))
        in_maps.append({
            "xTt": xTt,
            "wkv4": _bf16(wkv4_d),
            "wq4": _bf16(wq4_d),
            "woT": _bf16(woT_d),
            "cosT": cst["cosT"], "sinT": cst["sinT"],
            "rmat": cst["rmat"], "ident": cst["ident"],
            "tri": cst["tri"], "ksT": cst["ksT"],
            "ones128": cst["ones128"],
        })

    res = run_bass_kernel_spmd(nc, in_maps, core_ids=list(range(NCORES)),
                               trace=_trace)
    acc = res.results[0]["out"].astype(np.float64)
    for d in range(1, NCORES):
        acc += res.results[d]["out"].astype(np.float64)
    # out[n, tq, p, ti*512+f] -> out[128*(4tq+ti)+p, 512n+f]
    acc = acc.reshape(8, 4, 128, 4, 512)        # [n, tq, p, ti, f]
    outp = acc.transpose(1, 3, 2, 0, 4).reshape(S, D).astype(np.float32)
    outp = outp.reshape(1, S, D)
    if _trace:
        _CACHE["last_results"] = res
    return outp
